# revision 2
# baseline (speedup 1.0000x reference)
"""Single-head causal attention (B=4, S=4096, E=512, D=64) on 8 trn2 cores.

v3 (mixed precision): bf16 projections + bf16 scores (exact-ish), fp8
DoubleRow attend with hi/lo-split V, exp split across ACT and Pool.

Sharding: 8 cores = 4 batches x 2 query-interleave groups (as v1): core
(b, h) computes batch b, query tiles {h, h+2, ...}. Host permutes key
tiles per core (pair-swap for h=1) so the SPMD program is slot-identical;
exact causality in the diagonal band uses a 0/1 fp8 mask (Pool engine).

Datapath:
  - x^T, W in bf16; QKV projections as v1 (bf16 matmuls, f32 psum);
    K^T|V^T -> one bf16 SBUF tile (DVE bias-add copy), Q^T -> bf16.
  - scores = K_j^T . Q^T in bf16 per key block (f32 psum), exact.
  - softmax exp -> fp8e4m3 (e^(s/8 - 2); the -2 bias cancels in the
    division and keeps fp8 in range). ACT computes the band pairs + odd
    non-band pairs; Pool (gpsimd pow(e, y)) computes even non-band pairs
    from a DVE-staged copy (fused *0.125 - 2).
  - attend: fp8 DoubleRow, 2 instructions per 256-key block pair:
    vo_h = fp8(V) (+ ones column for the denominator), vo_l =
    fp8(V - vo_h) (the fp8 residual is small, so storing it unscaled in
    fp8 keeps ~0.1% absolute accuracy); both accumulate into one [80,512]
    psum = [numerator; denominator; pad]. Host divides + adds bv.
"""

import numpy as np
from contextlib import ExitStack

import concourse.mybir as mybir
import concourse.tile as tile
from concourse import bacc
from concourse.bass_utils import run_bass_kernel_spmd
from concourse.masks import make_identity

F32 = mybir.dt.float32
BF16 = mybir.dt.bfloat16
FP8 = mybir.dt.float8e4
AF = mybir.ActivationFunctionType
OP = mybir.AluOpType
DR = mybir.MatmulPerfMode.DoubleRow

B, S, E, D = 4, 4096, 512, 64
P = 128
EO = E // P
NT = S // P
KC = S // 512
QC = (S // 2) // 512
N_CORES = 8
ESC = 0.125
EBIAS = -2.0
VOW = 80  # fp8 DoubleRow stationary width (needs % 16 == 0)

_CACHE: dict = {}


def _build():
    nc = bacc.Bacc(
        "TRN2", target_bir_lowering=False, debug=False, num_devices=N_CORES
    )
    xkT = nc.dram_tensor("xkT", [E, S], BF16, kind="ExternalInput").ap()
    w3 = nc.dram_tensor("w3", [E, 3 * D], BF16, kind="ExternalInput").ap()
    consts = nc.dram_tensor("consts", [P, 522], F32, kind="ExternalInput").ap()
    outT = nc.dram_tensor("outT", [D + 1, S // 2], F32, kind="ExternalOutput").ap()

    with tile.TileContext(nc) as tc, ExitStack() as ctx:
        sb_const = ctx.enter_context(tc.tile_pool(name="const", bufs=1))
        sb_kv = ctx.enter_context(tc.tile_pool(name="kv", bufs=1))
        sb_xk = ctx.enter_context(tc.tile_pool(name="xk", bufs=6))
        sb_exp = ctx.enter_context(tc.tile_pool(name="exp", bufs=12))
        sb_sc = ctx.enter_context(tc.tile_pool(name="scb", bufs=4))
        sb_osb = ctx.enter_context(tc.tile_pool(name="osb", bufs=2))
        ps_misc = ctx.enter_context(tc.tile_pool(name="psm", bufs=1, space="PSUM"))
        ps_sc = ctx.enter_context(tc.tile_pool(name="pssc", bufs=3, space="PSUM"))
        ps_at = ctx.enter_context(tc.tile_pool(name="psat", bufs=1, space="PSUM"))
        
        # ---------------- constants ----------------
        w3t = sb_const.tile([P, EO, 3 * D], BF16)
        nc.scalar.dma_start(w3t[:], w3.rearrange("(eo p) d -> p eo d", p=P))
        cst = sb_const.tile([P, 522], F32)
        nc.scalar.dma_start(cst[:], consts)
        b2 = cst[:, 0:2]
        qb = cst[:, 2:514]
        kb = cst[:, 514:522]
        ebase = sb_const.tile([P, 1], F32)
        nc.gpsimd.memset(ebase[:], float(np.e))
        ebias = sb_const.tile([P, 1], F32)
        nc.gpsimd.memset(ebias[:], EBIAS)
        identF = sb_const.tile([P, P], F32)
        make_identity(nc, identF[:])
        identB = sb_const.tile([P, P], BF16)
        nc.vector.tensor_copy(identB[:], identF[:])
        bmask = sb_const.tile([P, 8, P], FP8)

        def build_masks():
            for jl in range(8):
                m = jl // 2
                qc0 = 128 * m
                nc.vector.tensor_tensor(
                    out=bmask[:, jl, :],
                    in0=qb[:, qc0 : qc0 + P],
                    in1=kb[:, jl : jl + 1].to_broadcast((P, P)),
                    op=OP.is_ge,
                )

        # ---------------- persistent state ----------------
        # kv rows 0:64 = K^T + bk, rows 64:128 = V^T (bf16)
        kv = sb_kv.tile([P, S], BF16)
        qts = sb_kv.tile([D, S // 2], BF16)
        # fp8 V blocks in [k, d] layout: hi + ones column, lo residual
        vo_h = sb_kv.tile([P, NT, VOW], FP8)
        vo_l = sb_kv.tile([P, NT, VOW], FP8)
        nc.gpsimd.memset(vo_h[:, :, D:VOW], 0.0)
        nc.gpsimd.memset(vo_h[:, :, D], 1.0)
        nc.gpsimd.memset(vo_l[:, :, D:VOW], 0.0)

        def phase_b(kc):
            xk = sb_xk.tile([P, EO, 512], BF16, tag="xk", name=f"xk{kc}")
            src = xkT[:, kc * 512 : (kc + 1) * 512].rearrange(
                "(eo p) k -> p eo k", p=P
            )
            if kc == 0:
                # two half-chunk pieces so the first projection starts early
                nc.sync.dma_start(xk[:, 0:2, :], src[:, 0:2, :])
                nc.sync.dma_start(xk[:, 2:4, :], src[:, 2:4, :])
            elif kc % 2 == 1:
                nc.scalar.dma_start(xk[:], src)
            else:
                nc.sync.dma_start(xk[:], src)
            pkv = ps_misc.tile([P, 512], F32, tag="ps", name=f"pkv{kc}")
            for eo in range(EO):
                nc.tensor.matmul(
                    pkv[:],
                    w3t[:, eo, D : 3 * D],
                    xk[:, eo, :],
                    start=(eo == 0),
                    stop=(eo == EO - 1),
                )
            nc.vector.tensor_tensor(
                out=kv[:, kc * 512 : (kc + 1) * 512],
                in0=pkv[:],
                in1=b2[:, 0:1].to_broadcast((P, 512)),
                op=OP.add,
            )
            return xk

        def phase_b_tr(kc):
            # V^T -> V transposes (4 key blocks, bf16), then split into
            # fp8 hi + fp8 residual lo
            pt = ps_misc.tile([P, 512], F32, tag="ps", name=f"pt{kc}")
            ptv = pt[:, 0:P].bitcast(BF16).rearrange(
                "p (b x) -> p b x", b=4
            )
            for bb in range(4):
                j = 4 * kc + bb
                nc.tensor.transpose(
                    ptv[:, bb, :],
                    kv[D:P, j * P : (j + 1) * P],
                    identB[D:P, D:P],
                )
            nc.vector.tensor_copy(
                vo_h[:, 4 * kc : 4 * kc + 4, 0:D], ptv[:]
            )
            nc.vector.tensor_tensor(
                out=vo_l[:, 4 * kc : 4 * kc + 4, 0:D],
                in0=ptv[:],
                in1=vo_h[:, 4 * kc : 4 * kc + 4, 0:D],
                op=OP.subtract,
            )

        pq_tiles = {}

        def phase_b_q_proj(c, xk_h, half):
            if half == 0:
                pq_tiles[c] = ps_sc.tile([P, 512], F32, tag="sc", name=f"pq{c}")
            pq = pq_tiles[c]
            for eo in range(EO):
                rhs = xk_h[:, eo, :].rearrange(
                    "p (t2 two x) -> p t2 two x", two=2, x=P
                )[:, :, 0, :]
                nc.tensor.matmul(
                    pq[0:D, half * 256 : (half + 1) * 256],
                    w3t[:, eo, 0:D],
                    rhs,
                    start=(eo == 0),
                    stop=(eo == EO - 1),
                )

        def phase_b_q_copy(c, half):
            pq = pq_tiles[c]
            nc.scalar.activation(
                qts[:, c * 512 + half * 256 : c * 512 + (half + 1) * 256],
                pq[0:D, half * 256 : (half + 1) * 256],
                AF.Identity,
                bias=b2[0:D, 1:2],
            )

        def phase_b_q_half(c, xk_h, half):
            phase_b_q_proj(c, xk_h, half)
            phase_b_q_copy(c, half)

        def phase_b_q(c, xk_a, xk_b):
            phase_b_q_proj(c, xk_a, 0)
            phase_b_q_proj(c, xk_b, 1)
            phase_b_q_copy(c, 0)
            phase_b_q_copy(c, 1)

        def emit_scores(psc, c, p2, qs):
            for ji, j in ((0, 2 * p2), (1, 2 * p2 + 1)):
                nc.tensor.matmul(
                    psc[:, 512 * ji + qs : 512 * ji + 512],
                    kv[0:D, j * P : (j + 1) * P],
                    qts[:, c * 512 + qs : (c + 1) * 512],
                    start=True,
                    stop=True,
                )

        def emit_exp_act(psc, eT, qe, width=512):
            psc_v = psc[:].rearrange("p (two x) -> p two x", x=512)
            eT_v = eT[:].rearrange("p (two x) -> p two x", x=512)
            nc.scalar.activation(
                eT_v[:, :, qe:width], psc_v[:, :, qe:width], AF.Exp,
                scale=ESC, bias=ebias[:, 0:1],
            )

        def emit_exp_pool(psc, eT, name):
            scb = sb_sc.tile([P, 1024], F32, tag="scb", name=name)
            for hh in range(2):
                sl = slice(512 * hh, 512 * hh + 512)
                nc.vector.tensor_scalar(
                    out=scb[:, sl], in0=psc[:, sl], scalar1=ESC, scalar2=EBIAS,
                    op0=OP.mult, op1=OP.add,
                )
                nc.gpsimd.tensor_tensor(
                    out=eT[:, sl], in0=ebase[:, 0:1].to_broadcast((P, 512)),
                    in1=scb[:, sl], op=OP.pow,
                )

        def emit_mask(eT, m, qe):
            eT_v = eT[:].rearrange("p (two x) -> p two x", x=512)
            nc.gpsimd.tensor_tensor(
                out=eT_v[:, :, qe : qe + P],
                in0=eT_v[:, :, qe : qe + P],
                in1=bmask[:, 2 * m : 2 * m + 2, :],
                op=OP.mult,
            )

        def emit_av(pat, eT, p2, qe, start, stop, width=512):
            eT_v = eT[:].rearrange("p (two x) -> p two x", x=512)
            nc.tensor.matmul(
                pat[:, qe:width],
                vo_h[:, 2 * p2 : 2 * p2 + 2, :],
                eT_v[:, :, qe:width],
                start=start,
                stop=False,
                perf_mode=DR,
            )
            nc.tensor.matmul(
                pat[:, qe:width],
                vo_l[:, 2 * p2 : 2 * p2 + 2, :],
                eT_v[:, :, qe:width],
                start=False,
                stop=stop,
                perf_mode=DR,
            )


        def phase_c(c, inject=None):
            AVLAG = 8 if c < QC - 1 else 4
            pat = ps_at.tile([VOW, 512], F32, tag="at", name=f"at{c}")
            npair = 4 * c + 4
            order = list(range(4 * c, npair)) + list(range(0, 4 * c))
            pend = []  # (eT, p2, qe) awaiting their AV emission
            navs = [0]

            def flush_av(last):
                eT, p2, qe = pend.pop(0)
                emit_av(pat, eT, p2, qe, start=(navs[0] == 0), stop=last)
                navs[0] += 1

            for idx, p2 in enumerate(order):
                if inject and idx in inject:
                    for fn in inject[idx]:
                        fn()
                m = p2 - 4 * c
                qs = 0 if m < 0 else min(128 * m, 256)
                qe = 0 if m < 0 else 128 * m
                psc = ps_sc.tile([P, 1024], F32, tag="sc", name=f"sc{c}_{p2}")
                emit_scores(psc, c, p2, qs)
                eT = sb_exp.tile([P, 1024], FP8, tag="eT", name=f"eT{c}_{p2}")
                if m < 0 and p2 % 2 == 0:
                    emit_exp_pool(psc, eT, f"scb{c}_{p2}")
                else:
                    emit_exp_act(psc, eT, qe)
                if m >= 0:
                    emit_mask(eT, m, qe)
                pend.append((eT, p2, qe))
                if len(pend) > AVLAG:
                    flush_av(False)
            while pend:
                flush_av(len(pend) == 1)
            osb = sb_osb.tile([D + 1, 512], F32, tag="osb", name=f"osb{c}")
            nc.vector.tensor_copy(osb[:], pat[0 : D + 1, :])
            nc.sync.dma_start(outT[:, c * 512 : (c + 1) * 512], osb[:])

        # ---------------- schedule ----------------
        xk_tiles = {}

        def mk(fn, *args):
            return lambda: fn(*args)

        def emit_b(kc):
            xk_tiles[kc] = phase_b(kc)

        def emit_q(c):
            phase_b_q(c, xk_tiles[2 * c], xk_tiles[2 * c + 1])

        def phase_c0_piece(pat0, m, h):
            qe = 128 * m
            a, b = max(qe, 256 * h), 256 * h + 256
            if b - a <= 0:
                return
            psc = ps_sc.tile([P, 1024], F32, tag="sc", name=f"s0_{m}_{h}")
            for ji, j in ((0, 2 * m), (1, 2 * m + 1)):
                nc.tensor.matmul(
                    psc[:, 512 * ji + a : 512 * ji + b],
                    kv[0:D, j * P : (j + 1) * P],
                    qts[:, a:b],
                    start=True,
                    stop=True,
                )
            eT = sb_exp.tile([P, 1024], FP8, tag="eT", name=f"e0_{m}_{h}")
            psc_v = psc[:].rearrange("p (two x) -> p two x", x=512)
            eT_v = eT[:].rearrange("p (two x) -> p two x", x=512)
            nc.scalar.activation(
                eT_v[:, :, a:b], psc_v[:, :, a:b], AF.Exp, scale=ESC,
                bias=ebias[:, 0:1],
            )
            if a <= qe < b:
                emit_mask(eT, m, qe)
            emit_av(pat0, eT, m, a, start=(m == 0), stop=(m == 3 and h == 1), width=b)

        # chunk-0 split head schedule
        emit_b(0)
        build_masks()
        phase_b_q_half(0, xk_tiles[0], 0)
        phase_b_tr(0)
        pat0 = ps_at.tile([VOW, 512], F32, tag="at", name="at0")
        phase_c0_piece(pat0, 0, 0)
        phase_c0_piece(pat0, 1, 0)
        emit_b(1)
        phase_b_q_half(0, xk_tiles[1], 1)
        phase_b_tr(1)
        phase_c0_piece(pat0, 0, 1)
        emit_b(2)
        phase_c0_piece(pat0, 1, 1)
        emit_b(3)
        phase_c0_piece(pat0, 2, 1)
        emit_q(1)
        phase_c0_piece(pat0, 3, 1)
        phase_b_tr(2)
        phase_b_tr(3)
        osb0 = sb_osb.tile([D + 1, 512], F32, tag="osb", name="osb0")
        nc.vector.tensor_copy(osb0[:], pat0[0 : D + 1, :])
        nc.sync.dma_start(outT[:, 0:512], osb0[:])

        inj_at = {1: [4, 6, 7, 7], 2: [5, 8, 11, 11]}
        for c in range(1, QC):
            if c < QC - 1:
                cn = c + 1
                pts = inj_at[c]
                items = [
                    [mk(emit_b, 2 * cn)],
                    [mk(emit_b, 2 * cn + 1)],
                    [mk(emit_q, cn)],
                    [mk(phase_b_tr, 2 * cn), mk(phase_b_tr, 2 * cn + 1)],
                ]
                inject = {}
                for pt, fns in zip(pts, items):
                    inject.setdefault(pt, []).extend(fns)
            else:
                inject = None
            phase_c(c, inject)

    nc.compile()
    return nc


def _stage_inputs(x, Wq, bq, Wk, bk, Wv, bv):
    import ml_dtypes

    x = np.asarray(x, dtype=np.float32)
    w3 = np.concatenate(
        [np.asarray(Wq), np.asarray(Wk), np.asarray(Wv)], axis=1
    ).astype(ml_dtypes.bfloat16)
    bias2 = np.zeros((P, 2), dtype=np.float32)
    bias2[0:D, 0] = np.asarray(bk, dtype=np.float32)
    bias2[0:D, 1] = np.asarray(bq, dtype=np.float32)

    qv = np.arange(512)
    in_maps = []
    for core in range(N_CORES):
        b, h = divmod(core, 2)
        g = np.arange(NT)
        if h == 1:
            g = g ^ 1
        xb = x[b].reshape(NT, P, E)[g]
        xkT_c = np.ascontiguousarray(
            xb.reshape(S, E).T.astype(ml_dtypes.bfloat16)
        )
        qpos = (P * (2 * (qv // P) + h) + (qv % P)).astype(np.float32)
        qband = np.ascontiguousarray(np.broadcast_to(qpos, (P, 512)))
        kk = np.arange(P)
        jl = np.arange(8)
        kband = (P * (jl[None, :] ^ h) + kk[:, None]).astype(np.float32)
        cst = np.empty((P, 522), dtype=np.float32)
        cst[:, 0:2] = bias2
        cst[:, 2:514] = qband
        cst[:, 514:522] = kband
        in_maps.append(
            {
                "xkT": xkT_c,
                "w3": w3,
                "consts": np.ascontiguousarray(cst),
            }
        )
    return in_maps


def _gather_output(results, bv):
    out = np.empty((B, S, D), dtype=np.float32)
    bv = np.asarray(bv, dtype=np.float32)
    tg = np.array([8 * c + 2 * si for c in range(QC) for si in range(4)])
    for core in range(N_CORES):
        b, h = divmod(core, 2)
        ot = results[core]["outT"]
        attn = ot[0:D] / ot[D : D + 1] + bv[:, None]
        blocks = attn.T.reshape(16, P, D)
        out.reshape(B, NT, P, D)[b, tg + h] = blocks
    return out


def kernel(x, Wq, bq, Wk, bk, Wv, bv):
    if "nc" not in _CACHE:
        _CACHE["nc"] = _build()
    nc = _CACHE["nc"]
    in_maps = _stage_inputs(x, Wq, bq, Wk, bk, Wv, bv)
    res = run_bass_kernel_spmd(nc, in_maps, core_ids=list(range(N_CORES)))
    return _gather_output(res.results, bv)


# revision 3
# speedup vs baseline: 1.0164x; 1.0164x over previous
"""Single-head causal attention (B=4, S=4096, E=512, D=64) on 8 trn2 cores.

v3 (mixed precision): bf16 projections + bf16 scores (exact-ish), fp8
DoubleRow attend with hi/lo-split V, exp split across ACT and Pool.

Sharding: 8 cores = 4 batches x 2 query-interleave groups (as v1): core
(b, h) computes batch b, query tiles {h, h+2, ...}. Host permutes key
tiles per core (pair-swap for h=1) so the SPMD program is slot-identical;
exact causality in the diagonal band uses a 0/1 fp8 mask (Pool engine).

Datapath:
  - x^T, W in bf16; QKV projections as v1 (bf16 matmuls, f32 psum);
    K^T|V^T -> one bf16 SBUF tile (DVE bias-add copy), Q^T -> bf16.
  - scores = K_j^T . Q^T in bf16 per key block (f32 psum), exact.
  - softmax exp -> fp8e4m3 (e^(s/8 - 2); the -2 bias cancels in the
    division and keeps fp8 in range). ACT computes the band pairs + odd
    non-band pairs; Pool (gpsimd pow(e, y)) computes even non-band pairs
    from a DVE-staged copy (fused *0.125 - 2).
  - attend: fp8 DoubleRow, 2 instructions per 256-key block pair:
    vo_h = fp8(V) (+ ones column for the denominator), vo_l =
    fp8(V - vo_h) (the fp8 residual is small, so storing it unscaled in
    fp8 keeps ~0.1% absolute accuracy); both accumulate into one [80,512]
    psum = [numerator; denominator; pad]. Host divides + adds bv.
"""

import numpy as np
from contextlib import ExitStack

import concourse.mybir as mybir
import concourse.tile as tile
from concourse import bacc
from concourse.bass_utils import run_bass_kernel_spmd
from concourse.masks import make_identity

F32 = mybir.dt.float32
BF16 = mybir.dt.bfloat16
FP8 = mybir.dt.float8e4
AF = mybir.ActivationFunctionType
OP = mybir.AluOpType
DR = mybir.MatmulPerfMode.DoubleRow

B, S, E, D = 4, 4096, 512, 64
P = 128
EO = E // P
NT = S // P
KC = S // 512
QC = (S // 2) // 512
N_CORES = 8
ESC = 0.125
EBIAS = -2.0
VOW = 80  # fp8 DoubleRow stationary width (needs % 16 == 0)

_CACHE: dict = {}


def _build():
    nc = bacc.Bacc(
        "TRN2", target_bir_lowering=False, debug=False, num_devices=N_CORES
    )
    xkT = nc.dram_tensor("xkT", [E, S], BF16, kind="ExternalInput").ap()
    w3 = nc.dram_tensor("w3", [E, 3 * D], BF16, kind="ExternalInput").ap()
    consts = nc.dram_tensor("consts", [P, 522], F32, kind="ExternalInput").ap()
    outT = nc.dram_tensor("outT", [D + 1, S // 2], F32, kind="ExternalOutput").ap()

    with tile.TileContext(nc) as tc, ExitStack() as ctx:
        sb_const = ctx.enter_context(tc.tile_pool(name="const", bufs=1))
        sb_kv = ctx.enter_context(tc.tile_pool(name="kv", bufs=1))
        sb_xk = ctx.enter_context(tc.tile_pool(name="xk", bufs=6))
        sb_exp = ctx.enter_context(tc.tile_pool(name="exp", bufs=12))
        sb_sc = ctx.enter_context(tc.tile_pool(name="scb", bufs=4))
        sb_osb = ctx.enter_context(tc.tile_pool(name="osb", bufs=2))
        ps_misc = ctx.enter_context(tc.tile_pool(name="psm", bufs=1, space="PSUM"))
        ps_sc = ctx.enter_context(tc.tile_pool(name="pssc", bufs=3, space="PSUM"))
        ps_at = ctx.enter_context(tc.tile_pool(name="psat", bufs=1, space="PSUM"))
        
        # ---------------- constants ----------------
        w3t = sb_const.tile([P, EO, 3 * D], BF16)
        nc.scalar.dma_start(w3t[:], w3.rearrange("(eo p) d -> p eo d", p=P))
        cst = sb_const.tile([P, 522], F32)
        nc.scalar.dma_start(cst[:], consts)
        b2 = cst[:, 0:2]
        qb = cst[:, 2:514]
        kb = cst[:, 514:522]
        ebase = sb_const.tile([P, 1], F32)
        nc.gpsimd.memset(ebase[:], float(np.e))
        ebias = sb_const.tile([P, 1], F32)
        nc.gpsimd.memset(ebias[:], EBIAS)
        identF = sb_const.tile([P, P], F32)
        make_identity(nc, identF[:])
        identB = sb_const.tile([P, P], BF16)
        nc.vector.tensor_copy(identB[:], identF[:])
        bmask = sb_const.tile([P, 8, P], FP8)

        def build_masks():
            for jl in range(8):
                m = jl // 2
                qc0 = 128 * m
                nc.vector.tensor_tensor(
                    out=bmask[:, jl, :],
                    in0=qb[:, qc0 : qc0 + P],
                    in1=kb[:, jl : jl + 1].to_broadcast((P, P)),
                    op=OP.is_ge,
                )

        # ---------------- persistent state ----------------
        # kv rows 0:64 = K^T + bk, rows 64:128 = V^T (bf16)
        kv = sb_kv.tile([P, S], BF16)
        qts = sb_kv.tile([D, S // 2], BF16)
        # fp8 V blocks in [k, d] layout: hi + ones column, lo residual
        vo_h = sb_kv.tile([P, NT, VOW], FP8)
        vo_l = sb_kv.tile([P, NT, VOW], FP8)
        nc.gpsimd.memset(vo_h[:, :, D:VOW], 0.0)
        nc.gpsimd.memset(vo_h[:, :, D], 1.0)
        nc.gpsimd.memset(vo_l[:, :, D:VOW], 0.0)

        def phase_b(kc):
            xk = sb_xk.tile([P, EO, 512], BF16, tag="xk", name=f"xk{kc}")
            src = xkT[:, kc * 512 : (kc + 1) * 512].rearrange(
                "(eo p) k -> p eo k", p=P
            )
            if kc == 0:
                # two half-chunk pieces so the first projection starts early
                nc.sync.dma_start(xk[:, 0:2, :], src[:, 0:2, :])
                nc.sync.dma_start(xk[:, 2:4, :], src[:, 2:4, :])
            elif kc % 2 == 1:
                nc.scalar.dma_start(xk[:], src)
            else:
                nc.sync.dma_start(xk[:], src)
            pkv = ps_misc.tile([P, 512], F32, tag="ps", name=f"pkv{kc}")
            for eo in range(EO):
                nc.tensor.matmul(
                    pkv[:],
                    w3t[:, eo, D : 3 * D],
                    xk[:, eo, :],
                    start=(eo == 0),
                    stop=(eo == EO - 1),
                )
            nc.vector.tensor_tensor(
                out=kv[:, kc * 512 : (kc + 1) * 512],
                in0=pkv[:],
                in1=b2[:, 0:1].to_broadcast((P, 512)),
                op=OP.add,
            )
            return xk

        def phase_b_tr(kc):
            # V^T -> V transposes (4 key blocks, bf16), then split into
            # fp8 hi + fp8 residual lo
            pt = ps_misc.tile([P, 512], F32, tag="ps", name=f"pt{kc}")
            ptv = pt[:, 0:P].bitcast(BF16).rearrange(
                "p (b x) -> p b x", b=4
            )
            for bb in range(4):
                j = 4 * kc + bb
                nc.tensor.transpose(
                    ptv[:, bb, :],
                    kv[D:P, j * P : (j + 1) * P],
                    identB[D:P, D:P],
                )
            nc.vector.tensor_copy(
                vo_h[:, 4 * kc : 4 * kc + 4, 0:D], ptv[:]
            )
            nc.vector.tensor_tensor(
                out=vo_l[:, 4 * kc : 4 * kc + 4, 0:D],
                in0=ptv[:],
                in1=vo_h[:, 4 * kc : 4 * kc + 4, 0:D],
                op=OP.subtract,
            )

        pq_tiles = {}

        def phase_b_q_proj(c, xk_h, half):
            if half == 0:
                pq_tiles[c] = ps_sc.tile([P, 512], F32, tag="sc", name=f"pq{c}")
            pq = pq_tiles[c]
            for eo in range(EO):
                rhs = xk_h[:, eo, :].rearrange(
                    "p (t2 two x) -> p t2 two x", two=2, x=P
                )[:, :, 0, :]
                nc.tensor.matmul(
                    pq[0:D, half * 256 : (half + 1) * 256],
                    w3t[:, eo, 0:D],
                    rhs,
                    start=(eo == 0),
                    stop=(eo == EO - 1),
                )

        def phase_b_q_copy(c, half):
            pq = pq_tiles[c]
            nc.scalar.activation(
                qts[:, c * 512 + half * 256 : c * 512 + (half + 1) * 256],
                pq[0:D, half * 256 : (half + 1) * 256],
                AF.Identity,
                bias=b2[0:D, 1:2],
            )

        def phase_b_q_half(c, xk_h, half):
            phase_b_q_proj(c, xk_h, half)
            phase_b_q_copy(c, half)

        def phase_b_q(c, xk_a, xk_b):
            phase_b_q_proj(c, xk_a, 0)
            phase_b_q_proj(c, xk_b, 1)
            phase_b_q_copy(c, 0)
            phase_b_q_copy(c, 1)

        def emit_scores(psc, c, p2, qs):
            for ji, j in ((0, 2 * p2), (1, 2 * p2 + 1)):
                nc.tensor.matmul(
                    psc[:, 512 * ji + qs : 512 * ji + 512],
                    kv[0:D, j * P : (j + 1) * P],
                    qts[:, c * 512 + qs : (c + 1) * 512],
                    start=True,
                    stop=True,
                )

        def emit_exp_act(psc, eT, qe, width=512):
            psc_v = psc[:].rearrange("p (two x) -> p two x", x=512)
            eT_v = eT[:].rearrange("p (two x) -> p two x", x=512)
            nc.scalar.activation(
                eT_v[:, :, qe:width], psc_v[:, :, qe:width], AF.Exp,
                scale=ESC, bias=ebias[:, 0:1],
            )

        def emit_exp_pool(psc, eT, name):
            scb = sb_sc.tile([P, 1024], F32, tag="scb", name=name)
            for hh in range(2):
                sl = slice(512 * hh, 512 * hh + 512)
                nc.vector.tensor_scalar(
                    out=scb[:, sl], in0=psc[:, sl], scalar1=ESC, scalar2=EBIAS,
                    op0=OP.mult, op1=OP.add,
                )
                nc.gpsimd.tensor_tensor(
                    out=eT[:, sl], in0=ebase[:, 0:1].to_broadcast((P, 512)),
                    in1=scb[:, sl], op=OP.pow,
                )

        def emit_mask(eT, m, qe):
            eT_v = eT[:].rearrange("p (two x) -> p two x", x=512)
            nc.gpsimd.tensor_tensor(
                out=eT_v[:, :, qe : qe + P],
                in0=eT_v[:, :, qe : qe + P],
                in1=bmask[:, 2 * m : 2 * m + 2, :],
                op=OP.mult,
            )

        def emit_av(pat, eT, p2, qe, start, stop, width=512):
            eT_v = eT[:].rearrange("p (two x) -> p two x", x=512)
            nc.tensor.matmul(
                pat[:, qe:width],
                vo_h[:, 2 * p2 : 2 * p2 + 2, :],
                eT_v[:, :, qe:width],
                start=start,
                stop=False,
                perf_mode=DR,
            )
            nc.tensor.matmul(
                pat[:, qe:width],
                vo_l[:, 2 * p2 : 2 * p2 + 2, :],
                eT_v[:, :, qe:width],
                start=False,
                stop=stop,
                perf_mode=DR,
            )


        def phase_c(c, inject=None):
            AVLAG = 8 if c < QC - 1 else 6
            pat = ps_at.tile([VOW, 512], F32, tag="at", name=f"at{c}")
            npair = 4 * c + 4
            order = list(range(4 * c, npair)) + list(range(0, 4 * c))
            pend = []  # (eT, p2, qe) awaiting their AV emission
            navs = [0]

            def flush_av(last):
                eT, p2, qe = pend.pop(0)
                emit_av(pat, eT, p2, qe, start=(navs[0] == 0), stop=last)
                navs[0] += 1

            for idx, p2 in enumerate(order):
                if inject and idx in inject:
                    for fn in inject[idx]:
                        fn()
                m = p2 - 4 * c
                qs = 0 if m < 0 else min(128 * m, 256)
                qe = 0 if m < 0 else 128 * m
                psc = ps_sc.tile([P, 1024], F32, tag="sc", name=f"sc{c}_{p2}")
                emit_scores(psc, c, p2, qs)
                eT = sb_exp.tile([P, 1024], FP8, tag="eT", name=f"eT{c}_{p2}")
                if m < 0 and p2 % 2 == 0 and not (c == 3 and p2 >= 8):
                    emit_exp_pool(psc, eT, f"scb{c}_{p2}")
                else:
                    emit_exp_act(psc, eT, qe)
                if m >= 0:
                    emit_mask(eT, m, qe)
                pend.append((eT, p2, qe))
                if len(pend) > AVLAG:
                    flush_av(False)
            while pend:
                flush_av(len(pend) == 1)
            osb = sb_osb.tile([D + 1, 512], F32, tag="osb", name=f"osb{c}")
            nc.vector.tensor_copy(osb[:], pat[0 : D + 1, :])
            nc.sync.dma_start(outT[:, c * 512 : (c + 1) * 512], osb[:])

        # ---------------- schedule ----------------
        xk_tiles = {}

        def mk(fn, *args):
            return lambda: fn(*args)

        def emit_b(kc):
            xk_tiles[kc] = phase_b(kc)

        def emit_q(c):
            phase_b_q(c, xk_tiles[2 * c], xk_tiles[2 * c + 1])

        def phase_c0_piece(pat0, m, h):
            qe = 128 * m
            a, b = max(qe, 256 * h), 256 * h + 256
            if b - a <= 0:
                return
            psc = ps_sc.tile([P, 1024], F32, tag="sc", name=f"s0_{m}_{h}")
            for ji, j in ((0, 2 * m), (1, 2 * m + 1)):
                nc.tensor.matmul(
                    psc[:, 512 * ji + a : 512 * ji + b],
                    kv[0:D, j * P : (j + 1) * P],
                    qts[:, a:b],
                    start=True,
                    stop=True,
                )
            eT = sb_exp.tile([P, 1024], FP8, tag="eT", name=f"e0_{m}_{h}")
            psc_v = psc[:].rearrange("p (two x) -> p two x", x=512)
            eT_v = eT[:].rearrange("p (two x) -> p two x", x=512)
            nc.scalar.activation(
                eT_v[:, :, a:b], psc_v[:, :, a:b], AF.Exp, scale=ESC,
                bias=ebias[:, 0:1],
            )
            if a <= qe < b:
                emit_mask(eT, m, qe)
            emit_av(pat0, eT, m, a, start=(m == 0), stop=(m == 3 and h == 1), width=b)

        # chunk-0 split head schedule
        emit_b(0)
        build_masks()
        phase_b_q_half(0, xk_tiles[0], 0)
        phase_b_tr(0)
        pat0 = ps_at.tile([VOW, 512], F32, tag="at", name="at0")
        phase_c0_piece(pat0, 0, 0)
        phase_c0_piece(pat0, 1, 0)
        emit_b(1)
        phase_b_q_half(0, xk_tiles[1], 1)
        phase_b_tr(1)
        phase_c0_piece(pat0, 0, 1)
        emit_b(2)
        phase_c0_piece(pat0, 1, 1)
        emit_b(3)
        phase_c0_piece(pat0, 2, 1)
        emit_q(1)
        phase_c0_piece(pat0, 3, 1)
        phase_b_tr(2)
        phase_b_tr(3)
        osb0 = sb_osb.tile([D + 1, 512], F32, tag="osb", name="osb0")
        nc.vector.tensor_copy(osb0[:], pat0[0 : D + 1, :])
        nc.sync.dma_start(outT[:, 0:512], osb0[:])

        inj_at = {1: [4, 6, 7, 7], 2: [5, 8, 11, 11]}
        for c in range(1, QC):
            if c < QC - 1:
                cn = c + 1
                pts = inj_at[c]
                items = [
                    [mk(emit_b, 2 * cn)],
                    [mk(emit_b, 2 * cn + 1)],
                    [mk(emit_q, cn)],
                    [mk(phase_b_tr, 2 * cn), mk(phase_b_tr, 2 * cn + 1)],
                ]
                inject = {}
                for pt, fns in zip(pts, items):
                    inject.setdefault(pt, []).extend(fns)
            else:
                inject = None
            phase_c(c, inject)

    nc.compile()
    return nc


def _stage_inputs(x, Wq, bq, Wk, bk, Wv, bv):
    import ml_dtypes

    x = np.asarray(x, dtype=np.float32)
    w3 = np.concatenate(
        [np.asarray(Wq), np.asarray(Wk), np.asarray(Wv)], axis=1
    ).astype(ml_dtypes.bfloat16)
    bias2 = np.zeros((P, 2), dtype=np.float32)
    bias2[0:D, 0] = np.asarray(bk, dtype=np.float32)
    bias2[0:D, 1] = np.asarray(bq, dtype=np.float32)

    qv = np.arange(512)
    in_maps = []
    for core in range(N_CORES):
        b, h = divmod(core, 2)
        g = np.arange(NT)
        if h == 1:
            g = g ^ 1
        xb = x[b].reshape(NT, P, E)[g]
        xkT_c = np.ascontiguousarray(
            xb.reshape(S, E).T.astype(ml_dtypes.bfloat16)
        )
        qpos = (P * (2 * (qv // P) + h) + (qv % P)).astype(np.float32)
        qband = np.ascontiguousarray(np.broadcast_to(qpos, (P, 512)))
        kk = np.arange(P)
        jl = np.arange(8)
        kband = (P * (jl[None, :] ^ h) + kk[:, None]).astype(np.float32)
        cst = np.empty((P, 522), dtype=np.float32)
        cst[:, 0:2] = bias2
        cst[:, 2:514] = qband
        cst[:, 514:522] = kband
        in_maps.append(
            {
                "xkT": xkT_c,
                "w3": w3,
                "consts": np.ascontiguousarray(cst),
            }
        )
    return in_maps


def _gather_output(results, bv):
    out = np.empty((B, S, D), dtype=np.float32)
    bv = np.asarray(bv, dtype=np.float32)
    tg = np.array([8 * c + 2 * si for c in range(QC) for si in range(4)])
    for core in range(N_CORES):
        b, h = divmod(core, 2)
        ot = results[core]["outT"]
        attn = ot[0:D] / ot[D : D + 1] + bv[:, None]
        blocks = attn.T.reshape(16, P, D)
        out.reshape(B, NT, P, D)[b, tg + h] = blocks
    return out


def kernel(x, Wq, bq, Wk, bk, Wv, bv):
    if "nc" not in _CACHE:
        _CACHE["nc"] = _build()
    nc = _CACHE["nc"]
    in_maps = _stage_inputs(x, Wq, bq, Wk, bk, Wv, bv)
    res = run_bass_kernel_spmd(nc, in_maps, core_ids=list(range(N_CORES)))
    return _gather_output(res.results, bv)


# revision 4
# speedup vs baseline: 1.0347x; 1.0180x over previous
"""Single-head causal attention (B=4, S=4096, E=512, D=64) on 8 trn2 cores.

v3 (mixed precision): bf16 projections + bf16 scores (exact-ish), fp8
DoubleRow attend with hi/lo-split V, exp split across ACT and Pool.

Sharding: 8 cores = 4 batches x 2 query-interleave groups (as v1): core
(b, h) computes batch b, query tiles {h, h+2, ...}. Host permutes key
tiles per core (pair-swap for h=1) so the SPMD program is slot-identical;
exact causality in the diagonal band uses a 0/1 fp8 mask (Pool engine).

Datapath:
  - x^T, W in bf16; QKV projections as v1 (bf16 matmuls, f32 psum);
    K^T|V^T -> one bf16 SBUF tile (DVE bias-add copy), Q^T -> bf16.
  - scores = K_j^T . Q^T in bf16 per key block (f32 psum), exact.
  - softmax exp -> fp8e4m3 (e^(s/8 - 2); the -2 bias cancels in the
    division and keeps fp8 in range). ACT computes the band pairs + odd
    non-band pairs; Pool (gpsimd pow(e, y)) computes even non-band pairs
    from a DVE-staged copy (fused *0.125 - 2).
  - attend: fp8 DoubleRow, 2 instructions per 256-key block pair:
    vo_h = fp8(V) (+ ones column for the denominator), vo_l =
    fp8(V - vo_h) (the fp8 residual is small, so storing it unscaled in
    fp8 keeps ~0.1% absolute accuracy); both accumulate into one [80,512]
    psum = [numerator; denominator; pad]. Host divides + adds bv.
"""

import numpy as np
from contextlib import ExitStack

import concourse.mybir as mybir
import concourse.tile as tile
from concourse import bacc
from concourse.bass_utils import run_bass_kernel_spmd
from concourse.masks import make_identity

F32 = mybir.dt.float32
BF16 = mybir.dt.bfloat16
FP8 = mybir.dt.float8e4
AF = mybir.ActivationFunctionType
OP = mybir.AluOpType
DR = mybir.MatmulPerfMode.DoubleRow

B, S, E, D = 4, 4096, 512, 64
P = 128
EO = E // P
NT = S // P
KC = S // 512
QC = (S // 2) // 512
N_CORES = 8
ESC = 0.125
EBIAS = -2.0
VOW = 80  # fp8 DoubleRow stationary width (needs % 16 == 0)

_CACHE: dict = {}


def _build():
    nc = bacc.Bacc(
        "TRN2", target_bir_lowering=False, debug=False, num_devices=N_CORES
    )
    xkT = nc.dram_tensor("xkT", [E, S], BF16, kind="ExternalInput").ap()
    w3 = nc.dram_tensor("w3", [E, 3 * D], BF16, kind="ExternalInput").ap()
    consts = nc.dram_tensor("consts", [P, 522], F32, kind="ExternalInput").ap()
    outT = nc.dram_tensor("outT", [D + 1, S // 2], F32, kind="ExternalOutput").ap()

    with tile.TileContext(nc) as tc, ExitStack() as ctx:
        sb_const = ctx.enter_context(tc.tile_pool(name="const", bufs=1))
        sb_kv = ctx.enter_context(tc.tile_pool(name="kv", bufs=1))
        sb_xk = ctx.enter_context(tc.tile_pool(name="xk", bufs=6))
        sb_exp = ctx.enter_context(tc.tile_pool(name="exp", bufs=12))
        sb_sc = ctx.enter_context(tc.tile_pool(name="scb", bufs=4))
        sb_osb = ctx.enter_context(tc.tile_pool(name="osb", bufs=2))
        ps_misc = ctx.enter_context(tc.tile_pool(name="psm", bufs=1, space="PSUM"))
        ps_sc = ctx.enter_context(tc.tile_pool(name="pssc", bufs=3, space="PSUM"))
        ps_at = ctx.enter_context(tc.tile_pool(name="psat", bufs=1, space="PSUM"))
        
        # ---------------- constants ----------------
        w3t = sb_const.tile([P, EO, 3 * D], BF16)
        nc.scalar.dma_start(w3t[:], w3.rearrange("(eo p) d -> p eo d", p=P))
        cst = sb_const.tile([P, 522], F32)
        nc.scalar.dma_start(cst[:], consts)
        b2 = cst[:, 0:2]
        qb = cst[:, 2:514]
        kb = cst[:, 514:522]
        ebase = sb_const.tile([P, 1], F32)
        nc.gpsimd.memset(ebase[:], float(np.e))
        ebias = sb_const.tile([P, 1], F32)
        nc.gpsimd.memset(ebias[:], EBIAS)
        identF = sb_const.tile([P, P], F32)
        make_identity(nc, identF[:])
        identB = sb_const.tile([P, P], BF16)
        nc.vector.tensor_copy(identB[:], identF[:])
        bmask = sb_const.tile([P, 8, P], FP8)

        def build_masks():
            for jl in range(8):
                m = jl // 2
                qc0 = 128 * m
                nc.vector.tensor_tensor(
                    out=bmask[:, jl, :],
                    in0=qb[:, qc0 : qc0 + P],
                    in1=kb[:, jl : jl + 1].to_broadcast((P, P)),
                    op=OP.is_ge,
                )

        # ---------------- persistent state ----------------
        # kv rows 0:64 = K^T + bk, rows 64:128 = V^T (bf16)
        kv = sb_kv.tile([P, S], BF16)
        qts = sb_kv.tile([D, S // 2], BF16)
        # fp8 V blocks in [k, d] layout: hi + ones column, lo residual
        vo_h = sb_kv.tile([P, NT, VOW], FP8)
        vo_l = sb_kv.tile([P, NT, VOW], FP8)
        nc.gpsimd.memset(vo_h[:, :, D:VOW], 0.0)
        nc.gpsimd.memset(vo_h[:, :, D], 1.0)
        nc.gpsimd.memset(vo_l[:, :, D:VOW], 0.0)

        def phase_b(kc):
            xk = sb_xk.tile([P, EO, 512], BF16, tag="xk", name=f"xk{kc}")
            src = xkT[:, kc * 512 : (kc + 1) * 512].rearrange(
                "(eo p) k -> p eo k", p=P
            )
            if kc == 0:
                # two half-chunk pieces so the first projection starts early
                nc.sync.dma_start(xk[:, 0:2, :], src[:, 0:2, :])
                nc.sync.dma_start(xk[:, 2:4, :], src[:, 2:4, :])
            elif kc % 2 == 1:
                nc.scalar.dma_start(xk[:], src)
            else:
                nc.sync.dma_start(xk[:], src)
            pkv = ps_misc.tile([P, 512], F32, tag="ps", name=f"pkv{kc}")
            for eo in range(EO):
                nc.tensor.matmul(
                    pkv[:],
                    w3t[:, eo, D : 3 * D],
                    xk[:, eo, :],
                    start=(eo == 0),
                    stop=(eo == EO - 1),
                )
            nc.vector.tensor_tensor(
                out=kv[:, kc * 512 : (kc + 1) * 512],
                in0=pkv[:],
                in1=b2[:, 0:1].to_broadcast((P, 512)),
                op=OP.add,
            )
            return xk

        def phase_b_tr(kc):
            # V^T -> V transposes (4 key blocks, bf16), then split into
            # fp8 hi + fp8 residual lo
            pt = ps_misc.tile([P, 512], F32, tag="ps", name=f"pt{kc}")
            ptv = pt[:, 0:P].bitcast(BF16).rearrange(
                "p (b x) -> p b x", b=4
            )
            for bb in range(4):
                j = 4 * kc + bb
                nc.tensor.transpose(
                    ptv[:, bb, :],
                    kv[D:P, j * P : (j + 1) * P],
                    identB[D:P, D:P],
                )
            nc.vector.tensor_copy(
                vo_h[:, 4 * kc : 4 * kc + 4, 0:D], ptv[:]
            )
            nc.vector.tensor_tensor(
                out=vo_l[:, 4 * kc : 4 * kc + 4, 0:D],
                in0=ptv[:],
                in1=vo_h[:, 4 * kc : 4 * kc + 4, 0:D],
                op=OP.subtract,
            )

        pq_tiles = {}

        def phase_b_q_proj(c, xk_h, half):
            if half == 0:
                pq_tiles[c] = ps_sc.tile([P, 512], F32, tag="sc", name=f"pq{c}")
            pq = pq_tiles[c]
            for eo in range(EO):
                rhs = xk_h[:, eo, :].rearrange(
                    "p (t2 two x) -> p t2 two x", two=2, x=P
                )[:, :, 0, :]
                nc.tensor.matmul(
                    pq[0:D, half * 256 : (half + 1) * 256],
                    w3t[:, eo, 0:D],
                    rhs,
                    start=(eo == 0),
                    stop=(eo == EO - 1),
                )

        def phase_b_q_copy(c, half):
            pq = pq_tiles[c]
            nc.scalar.activation(
                qts[:, c * 512 + half * 256 : c * 512 + (half + 1) * 256],
                pq[0:D, half * 256 : (half + 1) * 256],
                AF.Identity,
                bias=b2[0:D, 1:2],
            )

        def phase_b_q_half(c, xk_h, half):
            phase_b_q_proj(c, xk_h, half)
            phase_b_q_copy(c, half)

        def phase_b_q(c, xk_a, xk_b):
            phase_b_q_proj(c, xk_a, 0)
            phase_b_q_proj(c, xk_b, 1)
            phase_b_q_copy(c, 0)
            phase_b_q_copy(c, 1)

        def emit_scores(psc, c, p2, qs):
            for ji, j in ((0, 2 * p2), (1, 2 * p2 + 1)):
                nc.tensor.matmul(
                    psc[:, 512 * ji + qs : 512 * ji + 512],
                    kv[0:D, j * P : (j + 1) * P],
                    qts[:, c * 512 + qs : (c + 1) * 512],
                    start=True,
                    stop=True,
                )

        def emit_exp_act(psc, eT, qe, width=512):
            psc_v = psc[:].rearrange("p (two x) -> p two x", x=512)
            eT_v = eT[:].rearrange("p (two x) -> p two x", x=512)
            nc.scalar.activation(
                eT_v[:, :, qe:width], psc_v[:, :, qe:width], AF.Exp,
                scale=ESC, bias=ebias[:, 0:1],
            )

        def emit_exp_pool(psc, eT, name):
            scb = sb_sc.tile([P, 1024], F32, tag="scb", name=name)
            for hh in range(2):
                sl = slice(512 * hh, 512 * hh + 512)
                nc.vector.tensor_scalar(
                    out=scb[:, sl], in0=psc[:, sl], scalar1=ESC, scalar2=EBIAS,
                    op0=OP.mult, op1=OP.add,
                )
                nc.gpsimd.tensor_tensor(
                    out=eT[:, sl], in0=ebase[:, 0:1].to_broadcast((P, 512)),
                    in1=scb[:, sl], op=OP.pow,
                )

        def emit_mask(eT, m, qe):
            eT_v = eT[:].rearrange("p (two x) -> p two x", x=512)
            nc.gpsimd.tensor_tensor(
                out=eT_v[:, :, qe : qe + P],
                in0=eT_v[:, :, qe : qe + P],
                in1=bmask[:, 2 * m : 2 * m + 2, :],
                op=OP.mult,
            )

        def emit_av(pat, eT, p2, qe, start, stop, width=512):
            eT_v = eT[:].rearrange("p (two x) -> p two x", x=512)
            nc.tensor.matmul(
                pat[:, qe:width],
                vo_h[:, 2 * p2 : 2 * p2 + 2, :],
                eT_v[:, :, qe:width],
                start=start,
                stop=False,
                perf_mode=DR,
            )
            nc.tensor.matmul(
                pat[:, qe:width],
                vo_l[:, 2 * p2 : 2 * p2 + 2, :],
                eT_v[:, :, qe:width],
                start=False,
                stop=stop,
                perf_mode=DR,
            )


        def phase_c(c, inject=None):
            AVLAG = 8 if c < QC - 1 else 6
            pat = ps_at.tile([VOW, 512], F32, tag="at", name=f"at{c}")
            npair = 4 * c + 4
            order = list(range(4 * c, npair)) + list(range(0, 4 * c))
            pend = []  # (eT, p2, qe) awaiting their AV emission
            navs = [0]

            def flush_av(last):
                eT, p2, qe = pend.pop(0)
                emit_av(pat, eT, p2, qe, start=(navs[0] == 0), stop=last)
                navs[0] += 1

            for idx, p2 in enumerate(order):
                if inject and idx in inject:
                    for fn in inject[idx]:
                        fn()
                m = p2 - 4 * c
                qs = 0 if m < 0 else min(128 * m, 256)
                qe = 0 if m < 0 else 128 * m
                psc = ps_sc.tile([P, 1024], F32, tag="sc", name=f"sc{c}_{p2}")
                emit_scores(psc, c, p2, qs)
                eT = sb_exp.tile([P, 1024], FP8, tag="eT", name=f"eT{c}_{p2}")
                if m < 0 and p2 % 2 == 0 and not (c == 3 and p2 >= 8):
                    emit_exp_pool(psc, eT, f"scb{c}_{p2}")
                else:
                    emit_exp_act(psc, eT, qe)
                if m >= 0:
                    emit_mask(eT, m, qe)
                pend.append((eT, p2, qe))
                if len(pend) > AVLAG:
                    flush_av(False)
            while pend:
                flush_av(len(pend) == 1)
            osb = sb_osb.tile([D + 1, 512], F32, tag="osb", name=f"osb{c}")
            nc.scalar.copy(osb[:], pat[0 : D + 1, :])
            nc.sync.dma_start(outT[:, c * 512 : (c + 1) * 512], osb[:])

        # ---------------- schedule ----------------
        xk_tiles = {}

        def mk(fn, *args):
            return lambda: fn(*args)

        def emit_b(kc):
            xk_tiles[kc] = phase_b(kc)

        def emit_q(c):
            phase_b_q(c, xk_tiles[2 * c], xk_tiles[2 * c + 1])

        def phase_c0_piece(pat0, m, h):
            qe = 128 * m
            a, b = max(qe, 256 * h), 256 * h + 256
            if b - a <= 0:
                return
            psc = ps_sc.tile([P, 1024], F32, tag="sc", name=f"s0_{m}_{h}")
            for ji, j in ((0, 2 * m), (1, 2 * m + 1)):
                nc.tensor.matmul(
                    psc[:, 512 * ji + a : 512 * ji + b],
                    kv[0:D, j * P : (j + 1) * P],
                    qts[:, a:b],
                    start=True,
                    stop=True,
                )
            eT = sb_exp.tile([P, 1024], FP8, tag="eT", name=f"e0_{m}_{h}")
            psc_v = psc[:].rearrange("p (two x) -> p two x", x=512)
            eT_v = eT[:].rearrange("p (two x) -> p two x", x=512)
            nc.scalar.activation(
                eT_v[:, :, a:b], psc_v[:, :, a:b], AF.Exp, scale=ESC,
                bias=ebias[:, 0:1],
            )
            if a <= qe < b:
                emit_mask(eT, m, qe)
            emit_av(pat0, eT, m, a, start=(m == 0), stop=(m == 3 and h == 1), width=b)

        # chunk-0 split head schedule
        emit_b(0)
        build_masks()
        phase_b_q_half(0, xk_tiles[0], 0)
        phase_b_tr(0)
        pat0 = ps_at.tile([VOW, 512], F32, tag="at", name="at0")
        phase_c0_piece(pat0, 0, 0)
        phase_c0_piece(pat0, 1, 0)
        emit_b(1)
        phase_b_q_half(0, xk_tiles[1], 1)
        phase_b_tr(1)
        phase_c0_piece(pat0, 0, 1)
        emit_b(2)
        phase_c0_piece(pat0, 1, 1)
        emit_b(3)
        phase_c0_piece(pat0, 2, 1)
        emit_q(1)
        phase_c0_piece(pat0, 3, 1)
        phase_b_tr(2)
        phase_b_tr(3)
        osb0 = sb_osb.tile([D + 1, 512], F32, tag="osb", name="osb0")
        nc.scalar.copy(osb0[:], pat0[0 : D + 1, :])
        nc.sync.dma_start(outT[:, 0:512], osb0[:])

        inj_at = {1: [4, 6, 7, 7], 2: [5, 8, 11, 11]}
        for c in range(1, QC):
            if c < QC - 1:
                cn = c + 1
                pts = inj_at[c]
                items = [
                    [mk(emit_b, 2 * cn)],
                    [mk(emit_b, 2 * cn + 1)],
                    [mk(emit_q, cn)],
                    [mk(phase_b_tr, 2 * cn), mk(phase_b_tr, 2 * cn + 1)],
                ]
                inject = {}
                for pt, fns in zip(pts, items):
                    inject.setdefault(pt, []).extend(fns)
            else:
                inject = None
            phase_c(c, inject)

    nc.compile()
    return nc


def _stage_inputs(x, Wq, bq, Wk, bk, Wv, bv):
    import ml_dtypes

    x = np.asarray(x, dtype=np.float32)
    w3 = np.concatenate(
        [np.asarray(Wq), np.asarray(Wk), np.asarray(Wv)], axis=1
    ).astype(ml_dtypes.bfloat16)
    bias2 = np.zeros((P, 2), dtype=np.float32)
    bias2[0:D, 0] = np.asarray(bk, dtype=np.float32)
    bias2[0:D, 1] = np.asarray(bq, dtype=np.float32)

    qv = np.arange(512)
    in_maps = []
    for core in range(N_CORES):
        b, h = divmod(core, 2)
        g = np.arange(NT)
        if h == 1:
            g = g ^ 1
        xb = x[b].reshape(NT, P, E)[g]
        xkT_c = np.ascontiguousarray(
            xb.reshape(S, E).T.astype(ml_dtypes.bfloat16)
        )
        qpos = (P * (2 * (qv // P) + h) + (qv % P)).astype(np.float32)
        qband = np.ascontiguousarray(np.broadcast_to(qpos, (P, 512)))
        kk = np.arange(P)
        jl = np.arange(8)
        kband = (P * (jl[None, :] ^ h) + kk[:, None]).astype(np.float32)
        cst = np.empty((P, 522), dtype=np.float32)
        cst[:, 0:2] = bias2
        cst[:, 2:514] = qband
        cst[:, 514:522] = kband
        in_maps.append(
            {
                "xkT": xkT_c,
                "w3": w3,
                "consts": np.ascontiguousarray(cst),
            }
        )
    return in_maps


def _gather_output(results, bv):
    out = np.empty((B, S, D), dtype=np.float32)
    bv = np.asarray(bv, dtype=np.float32)
    tg = np.array([8 * c + 2 * si for c in range(QC) for si in range(4)])
    for core in range(N_CORES):
        b, h = divmod(core, 2)
        ot = results[core]["outT"]
        attn = ot[0:D] / ot[D : D + 1] + bv[:, None]
        blocks = attn.T.reshape(16, P, D)
        out.reshape(B, NT, P, D)[b, tg + h] = blocks
    return out


def kernel(x, Wq, bq, Wk, bk, Wv, bv):
    if "nc" not in _CACHE:
        _CACHE["nc"] = _build()
    nc = _CACHE["nc"]
    in_maps = _stage_inputs(x, Wq, bq, Wk, bk, Wv, bv)
    res = run_bass_kernel_spmd(nc, in_maps, core_ids=list(range(N_CORES)))
    return _gather_output(res.results, bv)


# revision 5
# speedup vs baseline: 1.0365x; 1.0018x over previous
"""Single-head causal attention (B=4, S=4096, E=512, D=64) on 8 trn2 cores.

v3 (mixed precision): bf16 projections + bf16 scores (exact-ish), fp8
DoubleRow attend with hi/lo-split V, exp split across ACT and Pool.

Sharding: 8 cores = 4 batches x 2 query-interleave groups (as v1): core
(b, h) computes batch b, query tiles {h, h+2, ...}. Host permutes key
tiles per core (pair-swap for h=1) so the SPMD program is slot-identical;
exact causality in the diagonal band uses a 0/1 fp8 mask (Pool engine).

Datapath:
  - x^T, W in bf16; QKV projections as v1 (bf16 matmuls, f32 psum);
    K^T|V^T -> one bf16 SBUF tile (DVE bias-add copy), Q^T -> bf16.
  - scores = K_j^T . Q^T in bf16 per key block (f32 psum), exact.
  - softmax exp -> fp8e4m3 (e^(s/8 - 2); the -2 bias cancels in the
    division and keeps fp8 in range). ACT computes the band pairs + odd
    non-band pairs; Pool (gpsimd pow(e, y)) computes even non-band pairs
    from a DVE-staged copy (fused *0.125 - 2).
  - attend: fp8 DoubleRow, 2 instructions per 256-key block pair:
    vo_h = fp8(V) (+ ones column for the denominator), vo_l =
    fp8(V - vo_h) (the fp8 residual is small, so storing it unscaled in
    fp8 keeps ~0.1% absolute accuracy); both accumulate into one [80,512]
    psum = [numerator; denominator; pad]. Host divides + adds bv.
"""

import numpy as np
from contextlib import ExitStack

import concourse.mybir as mybir
import concourse.tile as tile
from concourse import bacc
from concourse.bass_utils import run_bass_kernel_spmd
from concourse.masks import make_identity

F32 = mybir.dt.float32
BF16 = mybir.dt.bfloat16
FP8 = mybir.dt.float8e4
AF = mybir.ActivationFunctionType
OP = mybir.AluOpType
DR = mybir.MatmulPerfMode.DoubleRow

B, S, E, D = 4, 4096, 512, 64
P = 128
EO = E // P
NT = S // P
KC = S // 512
QC = (S // 2) // 512
N_CORES = 8
ESC = 0.125
EBIAS = -2.0
VOW = 80  # fp8 DoubleRow stationary width (needs % 16 == 0)

_CACHE: dict = {}


def _build():
    nc = bacc.Bacc(
        "TRN2", target_bir_lowering=False, debug=False, num_devices=N_CORES
    )
    xkT = nc.dram_tensor("xkT", [E, S], BF16, kind="ExternalInput").ap()
    w3 = nc.dram_tensor("w3", [E, 3 * D], BF16, kind="ExternalInput").ap()
    consts = nc.dram_tensor("consts", [P, 522], F32, kind="ExternalInput").ap()
    outT = nc.dram_tensor("outT", [D + 1, S // 2], F32, kind="ExternalOutput").ap()

    with tile.TileContext(nc) as tc, ExitStack() as ctx:
        sb_const = ctx.enter_context(tc.tile_pool(name="const", bufs=1))
        sb_kv = ctx.enter_context(tc.tile_pool(name="kv", bufs=1))
        sb_xk = ctx.enter_context(tc.tile_pool(name="xk", bufs=8))
        sb_exp = ctx.enter_context(tc.tile_pool(name="exp", bufs=12))
        sb_sc = ctx.enter_context(tc.tile_pool(name="scb", bufs=4))
        sb_osb = ctx.enter_context(tc.tile_pool(name="osb", bufs=2))
        ps_misc = ctx.enter_context(tc.tile_pool(name="psm", bufs=1, space="PSUM"))
        ps_sc = ctx.enter_context(tc.tile_pool(name="pssc", bufs=3, space="PSUM"))
        ps_at = ctx.enter_context(tc.tile_pool(name="psat", bufs=1, space="PSUM"))
        
        # ---------------- constants ----------------
        w3t = sb_const.tile([P, EO, 3 * D], BF16)
        nc.scalar.dma_start(w3t[:], w3.rearrange("(eo p) d -> p eo d", p=P))
        cst = sb_const.tile([P, 522], F32)
        nc.scalar.dma_start(cst[:], consts)
        b2 = cst[:, 0:2]
        qb = cst[:, 2:514]
        kb = cst[:, 514:522]
        ebase = sb_const.tile([P, 1], F32)
        nc.gpsimd.memset(ebase[:], float(np.e))
        ebias = sb_const.tile([P, 1], F32)
        nc.gpsimd.memset(ebias[:], EBIAS)
        identF = sb_const.tile([P, P], F32)
        make_identity(nc, identF[:])
        identB = sb_const.tile([P, P], BF16)
        nc.vector.tensor_copy(identB[:], identF[:])
        bmask = sb_const.tile([P, 8, P], FP8)

        def build_masks():
            for jl in range(8):
                m = jl // 2
                qc0 = 128 * m
                nc.vector.tensor_tensor(
                    out=bmask[:, jl, :],
                    in0=qb[:, qc0 : qc0 + P],
                    in1=kb[:, jl : jl + 1].to_broadcast((P, P)),
                    op=OP.is_ge,
                )

        # ---------------- persistent state ----------------
        # kv rows 0:64 = K^T + bk, rows 64:128 = V^T (bf16)
        kv = sb_kv.tile([P, S], BF16)
        qts = sb_kv.tile([D, S // 2], BF16)
        # fp8 V blocks in [k, d] layout: hi + ones column, lo residual
        vo_h = sb_kv.tile([P, NT, VOW], FP8)
        vo_l = sb_kv.tile([P, NT, VOW], FP8)
        nc.gpsimd.memset(vo_h[:, :, D:VOW], 0.0)
        nc.gpsimd.memset(vo_h[:, :, D], 1.0)
        nc.gpsimd.memset(vo_l[:, :, D:VOW], 0.0)

        def phase_b(kc):
            xk = sb_xk.tile([P, EO, 512], BF16, tag="xk", name=f"xk{kc}")
            src = xkT[:, kc * 512 : (kc + 1) * 512].rearrange(
                "(eo p) k -> p eo k", p=P
            )
            if kc == 0:
                # two half-chunk pieces so the first projection starts early
                nc.sync.dma_start(xk[:, 0:2, :], src[:, 0:2, :])
                nc.sync.dma_start(xk[:, 2:4, :], src[:, 2:4, :])
            elif kc % 2 == 1:
                nc.scalar.dma_start(xk[:], src)
            else:
                nc.sync.dma_start(xk[:], src)
            pkv = ps_misc.tile([P, 512], F32, tag="ps", name=f"pkv{kc}")
            for eo in range(EO):
                nc.tensor.matmul(
                    pkv[:],
                    w3t[:, eo, D : 3 * D],
                    xk[:, eo, :],
                    start=(eo == 0),
                    stop=(eo == EO - 1),
                )
            nc.vector.tensor_tensor(
                out=kv[:, kc * 512 : (kc + 1) * 512],
                in0=pkv[:],
                in1=b2[:, 0:1].to_broadcast((P, 512)),
                op=OP.add,
            )
            return xk

        def phase_b_tr(kc):
            # V^T -> V transposes (4 key blocks, bf16), then split into
            # fp8 hi + fp8 residual lo
            pt = ps_misc.tile([P, 512], F32, tag="ps", name=f"pt{kc}")
            ptv = pt[:, 0:P].bitcast(BF16).rearrange(
                "p (b x) -> p b x", b=4
            )
            for bb in range(4):
                j = 4 * kc + bb
                nc.tensor.transpose(
                    ptv[:, bb, :],
                    kv[D:P, j * P : (j + 1) * P],
                    identB[D:P, D:P],
                )
            nc.vector.tensor_copy(
                vo_h[:, 4 * kc : 4 * kc + 4, 0:D], ptv[:]
            )
            nc.vector.tensor_tensor(
                out=vo_l[:, 4 * kc : 4 * kc + 4, 0:D],
                in0=ptv[:],
                in1=vo_h[:, 4 * kc : 4 * kc + 4, 0:D],
                op=OP.subtract,
            )

        pq_tiles = {}

        def phase_b_q_proj(c, xk_h, half):
            if half == 0:
                pq_tiles[c] = ps_sc.tile([P, 512], F32, tag="sc", name=f"pq{c}")
            pq = pq_tiles[c]
            for eo in range(EO):
                rhs = xk_h[:, eo, :].rearrange(
                    "p (t2 two x) -> p t2 two x", two=2, x=P
                )[:, :, 0, :]
                nc.tensor.matmul(
                    pq[0:D, half * 256 : (half + 1) * 256],
                    w3t[:, eo, 0:D],
                    rhs,
                    start=(eo == 0),
                    stop=(eo == EO - 1),
                )

        def phase_b_q_copy(c, half):
            pq = pq_tiles[c]
            nc.scalar.activation(
                qts[:, c * 512 + half * 256 : c * 512 + (half + 1) * 256],
                pq[0:D, half * 256 : (half + 1) * 256],
                AF.Identity,
                bias=b2[0:D, 1:2],
            )

        def phase_b_q_half(c, xk_h, half):
            phase_b_q_proj(c, xk_h, half)
            phase_b_q_copy(c, half)

        def phase_b_q(c, xk_a, xk_b):
            phase_b_q_proj(c, xk_a, 0)
            phase_b_q_proj(c, xk_b, 1)
            phase_b_q_copy(c, 0)
            phase_b_q_copy(c, 1)

        def emit_scores(psc, c, p2, qs):
            for ji, j in ((0, 2 * p2), (1, 2 * p2 + 1)):
                nc.tensor.matmul(
                    psc[:, 512 * ji + qs : 512 * ji + 512],
                    kv[0:D, j * P : (j + 1) * P],
                    qts[:, c * 512 + qs : (c + 1) * 512],
                    start=True,
                    stop=True,
                )

        def emit_exp_act(psc, eT, qe, width=512):
            psc_v = psc[:].rearrange("p (two x) -> p two x", x=512)
            eT_v = eT[:].rearrange("p (two x) -> p two x", x=512)
            nc.scalar.activation(
                eT_v[:, :, qe:width], psc_v[:, :, qe:width], AF.Exp,
                scale=ESC, bias=ebias[:, 0:1],
            )

        def emit_exp_pool(psc, eT, name):
            scb = sb_sc.tile([P, 1024], F32, tag="scb", name=name)
            for hh in range(2):
                sl = slice(512 * hh, 512 * hh + 512)
                nc.vector.tensor_scalar(
                    out=scb[:, sl], in0=psc[:, sl], scalar1=ESC, scalar2=EBIAS,
                    op0=OP.mult, op1=OP.add,
                )
                nc.gpsimd.tensor_tensor(
                    out=eT[:, sl], in0=ebase[:, 0:1].to_broadcast((P, 512)),
                    in1=scb[:, sl], op=OP.pow,
                )

        def emit_mask(eT, m, qe):
            eT_v = eT[:].rearrange("p (two x) -> p two x", x=512)
            nc.gpsimd.tensor_tensor(
                out=eT_v[:, :, qe : qe + P],
                in0=eT_v[:, :, qe : qe + P],
                in1=bmask[:, 2 * m : 2 * m + 2, :],
                op=OP.mult,
            )

        def emit_av(pat, eT, p2, qe, start, stop, width=512):
            eT_v = eT[:].rearrange("p (two x) -> p two x", x=512)
            nc.tensor.matmul(
                pat[:, qe:width],
                vo_h[:, 2 * p2 : 2 * p2 + 2, :],
                eT_v[:, :, qe:width],
                start=start,
                stop=False,
                perf_mode=DR,
            )
            nc.tensor.matmul(
                pat[:, qe:width],
                vo_l[:, 2 * p2 : 2 * p2 + 2, :],
                eT_v[:, :, qe:width],
                start=False,
                stop=stop,
                perf_mode=DR,
            )


        def phase_c(c, inject=None):
            AVLAG = 8 if c < QC - 1 else 6
            pat = ps_at.tile([VOW, 512], F32, tag="at", name=f"at{c}")
            npair = 4 * c + 4
            order = list(range(4 * c, npair)) + list(range(0, 4 * c))
            pend = []  # (eT, p2, qe) awaiting their AV emission
            navs = [0]

            def flush_av(last):
                eT, p2, qe = pend.pop(0)
                emit_av(pat, eT, p2, qe, start=(navs[0] == 0), stop=last)
                navs[0] += 1

            for idx, p2 in enumerate(order):
                if inject and idx in inject:
                    for fn in inject[idx]:
                        fn()
                m = p2 - 4 * c
                qs = 0 if m < 0 else min(128 * m, 256)
                qe = 0 if m < 0 else 128 * m
                psc = ps_sc.tile([P, 1024], F32, tag="sc", name=f"sc{c}_{p2}")
                emit_scores(psc, c, p2, qs)
                eT = sb_exp.tile([P, 1024], FP8, tag="eT", name=f"eT{c}_{p2}")
                if m < 0 and p2 % 2 == 1 and not (c == 3 and p2 >= 8):
                    emit_exp_pool(psc, eT, f"scb{c}_{p2}")
                else:
                    emit_exp_act(psc, eT, qe)
                if m >= 0:
                    emit_mask(eT, m, qe)
                pend.append((eT, p2, qe))
                if len(pend) > AVLAG:
                    flush_av(False)
            while pend:
                flush_av(len(pend) == 1)
            osb = sb_osb.tile([D + 1, 512], F32, tag="osb", name=f"osb{c}")
            nc.scalar.copy(osb[:], pat[0 : D + 1, :])
            nc.sync.dma_start(outT[:, c * 512 : (c + 1) * 512], osb[:])

        # ---------------- schedule ----------------
        xk_tiles = {}

        def mk(fn, *args):
            return lambda: fn(*args)

        def emit_b(kc):
            xk_tiles[kc] = phase_b(kc)

        def emit_q(c):
            phase_b_q(c, xk_tiles[2 * c], xk_tiles[2 * c + 1])

        def phase_c0_piece(pat0, m, h):
            qe = 128 * m
            a, b = max(qe, 256 * h), 256 * h + 256
            if b - a <= 0:
                return
            psc = ps_sc.tile([P, 1024], F32, tag="sc", name=f"s0_{m}_{h}")
            for ji, j in ((0, 2 * m), (1, 2 * m + 1)):
                nc.tensor.matmul(
                    psc[:, 512 * ji + a : 512 * ji + b],
                    kv[0:D, j * P : (j + 1) * P],
                    qts[:, a:b],
                    start=True,
                    stop=True,
                )
            eT = sb_exp.tile([P, 1024], FP8, tag="eT", name=f"e0_{m}_{h}")
            psc_v = psc[:].rearrange("p (two x) -> p two x", x=512)
            eT_v = eT[:].rearrange("p (two x) -> p two x", x=512)
            nc.scalar.activation(
                eT_v[:, :, a:b], psc_v[:, :, a:b], AF.Exp, scale=ESC,
                bias=ebias[:, 0:1],
            )
            if a <= qe < b:
                emit_mask(eT, m, qe)
            emit_av(pat0, eT, m, a, start=(m == 0), stop=(m == 3 and h == 1), width=b)

        # chunk-0 split head schedule
        emit_b(0)
        build_masks()
        phase_b_q_half(0, xk_tiles[0], 0)
        phase_b_tr(0)
        pat0 = ps_at.tile([VOW, 512], F32, tag="at", name="at0")
        phase_c0_piece(pat0, 0, 0)
        phase_c0_piece(pat0, 1, 0)
        emit_b(1)
        phase_b_q_half(0, xk_tiles[1], 1)
        phase_b_tr(1)
        phase_c0_piece(pat0, 0, 1)
        emit_b(2)
        phase_c0_piece(pat0, 1, 1)
        emit_b(3)
        phase_c0_piece(pat0, 2, 1)
        emit_q(1)
        phase_c0_piece(pat0, 3, 1)
        phase_b_tr(2)
        phase_b_tr(3)
        osb0 = sb_osb.tile([D + 1, 512], F32, tag="osb", name="osb0")
        nc.scalar.copy(osb0[:], pat0[0 : D + 1, :])
        nc.sync.dma_start(outT[:, 0:512], osb0[:])

        inj_at = {1: [4, 6, 7, 7], 2: [5, 8, 11, 11]}
        for c in range(1, QC):
            if c < QC - 1:
                cn = c + 1
                pts = inj_at[c]
                items = [
                    [mk(emit_b, 2 * cn)],
                    [mk(emit_b, 2 * cn + 1)],
                    [mk(emit_q, cn)],
                    [mk(phase_b_tr, 2 * cn), mk(phase_b_tr, 2 * cn + 1)],
                ]
                inject = {}
                for pt, fns in zip(pts, items):
                    inject.setdefault(pt, []).extend(fns)
            else:
                inject = None
            phase_c(c, inject)

    nc.compile()
    return nc


def _stage_inputs(x, Wq, bq, Wk, bk, Wv, bv):
    import ml_dtypes

    x = np.asarray(x, dtype=np.float32)
    w3 = np.concatenate(
        [np.asarray(Wq), np.asarray(Wk), np.asarray(Wv)], axis=1
    ).astype(ml_dtypes.bfloat16)
    bias2 = np.zeros((P, 2), dtype=np.float32)
    bias2[0:D, 0] = np.asarray(bk, dtype=np.float32)
    bias2[0:D, 1] = np.asarray(bq, dtype=np.float32)

    qv = np.arange(512)
    in_maps = []
    for core in range(N_CORES):
        b, h = divmod(core, 2)
        g = np.arange(NT)
        if h == 1:
            g = g ^ 1
        xb = x[b].reshape(NT, P, E)[g]
        xkT_c = np.ascontiguousarray(
            xb.reshape(S, E).T.astype(ml_dtypes.bfloat16)
        )
        qpos = (P * (2 * (qv // P) + h) + (qv % P)).astype(np.float32)
        qband = np.ascontiguousarray(np.broadcast_to(qpos, (P, 512)))
        kk = np.arange(P)
        jl = np.arange(8)
        kband = (P * (jl[None, :] ^ h) + kk[:, None]).astype(np.float32)
        cst = np.empty((P, 522), dtype=np.float32)
        cst[:, 0:2] = bias2
        cst[:, 2:514] = qband
        cst[:, 514:522] = kband
        in_maps.append(
            {
                "xkT": xkT_c,
                "w3": w3,
                "consts": np.ascontiguousarray(cst),
            }
        )
    return in_maps


def _gather_output(results, bv):
    out = np.empty((B, S, D), dtype=np.float32)
    bv = np.asarray(bv, dtype=np.float32)
    tg = np.array([8 * c + 2 * si for c in range(QC) for si in range(4)])
    for core in range(N_CORES):
        b, h = divmod(core, 2)
        ot = results[core]["outT"]
        attn = ot[0:D] / ot[D : D + 1] + bv[:, None]
        blocks = attn.T.reshape(16, P, D)
        out.reshape(B, NT, P, D)[b, tg + h] = blocks
    return out


def kernel(x, Wq, bq, Wk, bk, Wv, bv):
    if "nc" not in _CACHE:
        _CACHE["nc"] = _build()
    nc = _CACHE["nc"]
    in_maps = _stage_inputs(x, Wq, bq, Wk, bk, Wv, bv)
    res = run_bass_kernel_spmd(nc, in_maps, core_ids=list(range(N_CORES)))
    return _gather_output(res.results, bv)


# revision 6
# speedup vs baseline: 1.0529x; 1.0158x over previous
"""Single-head causal attention (B=4, S=4096, E=512, D=64) on 8 trn2 cores.

v3 (mixed precision): bf16 projections + bf16 scores (exact-ish), fp8
DoubleRow attend with hi/lo-split V, exp split across ACT and Pool.

Sharding: 8 cores = 4 batches x 2 query-interleave groups (as v1): core
(b, h) computes batch b, query tiles {h, h+2, ...}. Host permutes key
tiles per core (pair-swap for h=1) so the SPMD program is slot-identical;
exact causality in the diagonal band uses a 0/1 fp8 mask (Pool engine).

Datapath:
  - x^T, W in bf16; QKV projections as v1 (bf16 matmuls, f32 psum);
    K^T|V^T -> one bf16 SBUF tile (DVE bias-add copy), Q^T -> bf16.
  - scores = K_j^T . Q^T in bf16 per key block (f32 psum), exact.
  - softmax exp -> fp8e4m3 (e^(s/8 - 2); the -2 bias cancels in the
    division and keeps fp8 in range). ACT computes the band pairs + odd
    non-band pairs; Pool (gpsimd pow(e, y)) computes even non-band pairs
    from a DVE-staged copy (fused *0.125 - 2).
  - attend: fp8 DoubleRow, 2 instructions per 256-key block pair:
    vo_h = fp8(V) (+ ones column for the denominator), vo_l =
    fp8(V - vo_h) (the fp8 residual is small, so storing it unscaled in
    fp8 keeps ~0.1% absolute accuracy); both accumulate into one [80,512]
    psum = [numerator; denominator; pad]. Host divides + adds bv.
"""

import numpy as np
from contextlib import ExitStack

import concourse.mybir as mybir
import concourse.tile as tile
from concourse import bacc
from concourse.bass_utils import run_bass_kernel_spmd
from concourse.masks import make_identity

F32 = mybir.dt.float32
BF16 = mybir.dt.bfloat16
FP8 = mybir.dt.float8e4
AF = mybir.ActivationFunctionType
OP = mybir.AluOpType
DR = mybir.MatmulPerfMode.DoubleRow

B, S, E, D = 4, 4096, 512, 64
P = 128
EO = E // P
NT = S // P
KC = S // 512
QC = (S // 2) // 512
N_CORES = 8
ESC = 0.125
EBIAS = -2.0
VOW = 80  # fp8 DoubleRow stationary width (needs % 16 == 0)

_CACHE: dict = {}


def _build():
    nc = bacc.Bacc(
        "TRN2", target_bir_lowering=False, debug=False, num_devices=N_CORES
    )
    xkT = nc.dram_tensor("xkT", [E, S], BF16, kind="ExternalInput").ap()
    w3 = nc.dram_tensor("w3", [E, 3 * D], BF16, kind="ExternalInput").ap()
    consts = nc.dram_tensor("consts", [P, 522], F32, kind="ExternalInput").ap()
    outT = nc.dram_tensor("outT", [D + 1, S // 2], F32, kind="ExternalOutput").ap()

    with tile.TileContext(nc) as tc, ExitStack() as ctx:
        sb_const = ctx.enter_context(tc.tile_pool(name="const", bufs=1))
        sb_kv = ctx.enter_context(tc.tile_pool(name="kv", bufs=1))
        sb_xk = ctx.enter_context(tc.tile_pool(name="xk", bufs=8))
        sb_exp = ctx.enter_context(tc.tile_pool(name="exp", bufs=12))
        sb_sc = ctx.enter_context(tc.tile_pool(name="scb", bufs=4))
        sb_osb = ctx.enter_context(tc.tile_pool(name="osb", bufs=2))
        ps_misc = ctx.enter_context(tc.tile_pool(name="psm", bufs=1, space="PSUM"))
        ps_sc = ctx.enter_context(tc.tile_pool(name="pssc", bufs=3, space="PSUM"))
        ps_at = ctx.enter_context(tc.tile_pool(name="psat", bufs=1, space="PSUM"))
        
        # ---------------- constants ----------------
        w3t = sb_const.tile([P, EO, 3 * D], BF16)
        nc.scalar.dma_start(w3t[:], w3.rearrange("(eo p) d -> p eo d", p=P))
        cst = sb_const.tile([P, 522], F32)
        nc.scalar.dma_start(cst[:], consts)
        b2 = cst[:, 0:2]
        qb = cst[:, 2:514]
        kb = cst[:, 514:522]
        ebase = sb_const.tile([P, 1], F32)
        nc.gpsimd.memset(ebase[:], float(np.e))
        ebias = sb_const.tile([P, 1], F32)
        nc.gpsimd.memset(ebias[:], EBIAS)
        identF = sb_const.tile([P, P], F32)
        make_identity(nc, identF[:])
        identB = sb_const.tile([P, P], BF16)
        nc.vector.tensor_copy(identB[:], identF[:])
        bmask = sb_const.tile([P, 8, P], FP8)

        def build_masks():
            for jl in range(8):
                m = jl // 2
                qc0 = 128 * m
                nc.vector.tensor_tensor(
                    out=bmask[:, jl, :],
                    in0=qb[:, qc0 : qc0 + P],
                    in1=kb[:, jl : jl + 1].to_broadcast((P, P)),
                    op=OP.is_ge,
                )

        # ---------------- persistent state ----------------
        # kv rows 0:64 = K^T + bk, rows 64:128 = V^T (bf16)
        kv = sb_kv.tile([P, S], BF16)
        qts = sb_kv.tile([D, S // 2], BF16)
        # fp8 V blocks in [k, d] layout: hi + ones column, lo residual
        vo_h = sb_kv.tile([P, NT, VOW], FP8)
        vo_l = sb_kv.tile([P, NT, VOW], FP8)
        nc.gpsimd.memset(vo_h[:, :, D:VOW], 0.0)
        nc.gpsimd.memset(vo_h[:, :, D], 1.0)
        nc.gpsimd.memset(vo_l[:, :, D:VOW], 0.0)

        def phase_b(kc):
            xk = sb_xk.tile([P, EO, 512], BF16, tag="xk", name=f"xk{kc}")
            src = xkT[:, kc * 512 : (kc + 1) * 512].rearrange(
                "(eo p) k -> p eo k", p=P
            )
            if kc == 0:
                # two half-chunk pieces so the first projection starts early
                nc.sync.dma_start(xk[:, 0:2, :], src[:, 0:2, :])
                nc.sync.dma_start(xk[:, 2:4, :], src[:, 2:4, :])
            elif kc % 2 == 1:
                nc.scalar.dma_start(xk[:], src)
            else:
                nc.sync.dma_start(xk[:], src)
            pkv = ps_misc.tile([P, 512], F32, tag="ps", name=f"pkv{kc}")
            for eo in range(EO):
                nc.tensor.matmul(
                    pkv[:],
                    w3t[:, eo, D : 3 * D],
                    xk[:, eo, :],
                    start=(eo == 0),
                    stop=(eo == EO - 1),
                )
            nc.vector.tensor_tensor(
                out=kv[:, kc * 512 : (kc + 1) * 512],
                in0=pkv[:],
                in1=b2[:, 0:1].to_broadcast((P, 512)),
                op=OP.add,
            )
            return xk

        def phase_b_tr(kc):
            # V^T -> V transposes (4 key blocks, bf16), then split into
            # fp8 hi + fp8 residual lo
            pt = ps_misc.tile([P, 512], F32, tag="ps", name=f"pt{kc}")
            ptv = pt[:, 0:P].bitcast(BF16).rearrange(
                "p (b x) -> p b x", b=4
            )
            for bb in range(4):
                j = 4 * kc + bb
                nc.tensor.transpose(
                    ptv[:, bb, :],
                    kv[D:P, j * P : (j + 1) * P],
                    identB[D:P, D:P],
                )
            nc.vector.tensor_copy(
                vo_h[:, 4 * kc : 4 * kc + 4, 0:D], ptv[:]
            )
            nc.vector.tensor_tensor(
                out=vo_l[:, 4 * kc : 4 * kc + 4, 0:D],
                in0=ptv[:],
                in1=vo_h[:, 4 * kc : 4 * kc + 4, 0:D],
                op=OP.subtract,
            )

        pq_tiles = {}

        def phase_b_q_proj(c, xk_h, half):
            if half == 0:
                pq_tiles[c] = ps_sc.tile([P, 512], F32, tag="sc", name=f"pq{c}")
            pq = pq_tiles[c]
            for eo in range(EO):
                rhs = xk_h[:, eo, :].rearrange(
                    "p (t2 two x) -> p t2 two x", two=2, x=P
                )[:, :, 0, :]
                nc.tensor.matmul(
                    pq[0:D, half * 256 : (half + 1) * 256],
                    w3t[:, eo, 0:D],
                    rhs,
                    start=(eo == 0),
                    stop=(eo == EO - 1),
                )

        def phase_b_q_copy(c, half):
            pq = pq_tiles[c]
            nc.scalar.activation(
                qts[:, c * 512 + half * 256 : c * 512 + (half + 1) * 256],
                pq[0:D, half * 256 : (half + 1) * 256],
                AF.Identity,
                bias=b2[0:D, 1:2],
            )

        def phase_b_q_half(c, xk_h, half):
            phase_b_q_proj(c, xk_h, half)
            phase_b_q_copy(c, half)

        def phase_b_q(c, xk_a, xk_b):
            phase_b_q_proj(c, xk_a, 0)
            phase_b_q_proj(c, xk_b, 1)
            phase_b_q_copy(c, 0)
            phase_b_q_copy(c, 1)

        def emit_scores(psc, c, p2, qs):
            for ji, j in ((0, 2 * p2), (1, 2 * p2 + 1)):
                nc.tensor.matmul(
                    psc[:, 512 * ji + qs : 512 * ji + 512],
                    kv[0:D, j * P : (j + 1) * P],
                    qts[:, c * 512 + qs : (c + 1) * 512],
                    start=True,
                    stop=True,
                )

        def emit_exp_act(psc, eT, qe, width=512):
            psc_v = psc[:].rearrange("p (two x) -> p two x", x=512)
            eT_v = eT[:].rearrange("p (two x) -> p two x", x=512)
            nc.scalar.activation(
                eT_v[:, :, qe:width], psc_v[:, :, qe:width], AF.Exp,
                scale=ESC, bias=ebias[:, 0:1],
            )

        def emit_exp_pool(psc, eT, name):
            scb = sb_sc.tile([P, 1024], F32, tag="scb", name=name)
            for hh in range(2):
                sl = slice(512 * hh, 512 * hh + 512)
                nc.vector.tensor_scalar(
                    out=scb[:, sl], in0=psc[:, sl], scalar1=ESC, scalar2=EBIAS,
                    op0=OP.mult, op1=OP.add,
                )
                nc.gpsimd.tensor_tensor(
                    out=eT[:, sl], in0=ebase[:, 0:1].to_broadcast((P, 512)),
                    in1=scb[:, sl], op=OP.pow,
                )

        def emit_mask(eT, m, qe):
            eT_v = eT[:].rearrange("p (two x) -> p two x", x=512)
            nc.gpsimd.tensor_tensor(
                out=eT_v[:, :, qe : qe + P],
                in0=eT_v[:, :, qe : qe + P],
                in1=bmask[:, 2 * m : 2 * m + 2, :],
                op=OP.mult,
            )

        def emit_av(pat, eT, p2, qe, start, stop, width=512):
            eT_v = eT[:].rearrange("p (two x) -> p two x", x=512)
            nc.tensor.matmul(
                pat[:, qe:width],
                vo_h[:, 2 * p2 : 2 * p2 + 2, :],
                eT_v[:, :, qe:width],
                start=start,
                stop=False,
                perf_mode=DR,
            )
            nc.tensor.matmul(
                pat[:, qe:width],
                vo_l[:, 2 * p2 : 2 * p2 + 2, :],
                eT_v[:, :, qe:width],
                start=False,
                stop=stop,
                perf_mode=DR,
            )


        def phase_c(c, inject=None):
            AVLAG = 8 if c < QC - 1 else 6
            pat = ps_at.tile([VOW, 512], F32, tag="at", name=f"at{c}")
            npair = 4 * c + 4
            order = list(range(4 * c, npair)) + list(range(0, 4 * c))
            pend = []  # (eT, p2, qe) awaiting their AV emission
            navs = [0]

            def flush_av(last):
                eT, p2, qe = pend.pop(0)
                emit_av(pat, eT, p2, qe, start=(navs[0] == 0), stop=last)
                navs[0] += 1

            for idx, p2 in enumerate(order):
                if inject and idx in inject:
                    for fn in inject[idx]:
                        fn()
                m = p2 - 4 * c
                qs = 0 if m < 0 else min(128 * m, 256)
                qe = 0 if m < 0 else 128 * m
                psc = ps_sc.tile([P, 1024], F32, tag="sc", name=f"sc{c}_{p2}")
                emit_scores(psc, c, p2, qs)
                eT = sb_exp.tile([P, 1024], FP8, tag="eT", name=f"eT{c}_{p2}")
                if m < 0 and p2 % 2 == 1 and not (c == 3 and p2 >= 10):
                    emit_exp_pool(psc, eT, f"scb{c}_{p2}")
                else:
                    emit_exp_act(psc, eT, qe)
                if m >= 0:
                    emit_mask(eT, m, qe)
                pend.append((eT, p2, qe))
                if len(pend) > AVLAG:
                    flush_av(False)
            while pend:
                flush_av(len(pend) == 1)
            osb = sb_osb.tile([D + 1, 512], F32, tag="osb", name=f"osb{c}")
            nc.scalar.copy(osb[:], pat[0 : D + 1, :])
            nc.sync.dma_start(outT[:, c * 512 : (c + 1) * 512], osb[:])

        # ---------------- schedule ----------------
        xk_tiles = {}

        def mk(fn, *args):
            return lambda: fn(*args)

        def emit_b(kc):
            xk_tiles[kc] = phase_b(kc)

        def emit_q(c):
            phase_b_q(c, xk_tiles[2 * c], xk_tiles[2 * c + 1])

        def phase_c0_piece(pat0, m, h):
            qe = 128 * m
            a, b = max(qe, 256 * h), 256 * h + 256
            if b - a <= 0:
                return
            psc = ps_sc.tile([P, 1024], F32, tag="sc", name=f"s0_{m}_{h}")
            for ji, j in ((0, 2 * m), (1, 2 * m + 1)):
                nc.tensor.matmul(
                    psc[:, 512 * ji + a : 512 * ji + b],
                    kv[0:D, j * P : (j + 1) * P],
                    qts[:, a:b],
                    start=True,
                    stop=True,
                )
            eT = sb_exp.tile([P, 1024], FP8, tag="eT", name=f"e0_{m}_{h}")
            psc_v = psc[:].rearrange("p (two x) -> p two x", x=512)
            eT_v = eT[:].rearrange("p (two x) -> p two x", x=512)
            nc.scalar.activation(
                eT_v[:, :, a:b], psc_v[:, :, a:b], AF.Exp, scale=ESC,
                bias=ebias[:, 0:1],
            )
            if a <= qe < b:
                emit_mask(eT, m, qe)
            emit_av(pat0, eT, m, a, start=(m == 0), stop=(m == 3 and h == 1), width=b)

        # chunk-0 split head schedule
        emit_b(0)
        build_masks()
        phase_b_q_half(0, xk_tiles[0], 0)
        phase_b_tr(0)
        pat0 = ps_at.tile([VOW, 512], F32, tag="at", name="at0")
        phase_c0_piece(pat0, 0, 0)
        phase_c0_piece(pat0, 1, 0)
        emit_b(1)
        phase_b_q_half(0, xk_tiles[1], 1)
        phase_b_tr(1)
        phase_c0_piece(pat0, 0, 1)
        emit_b(2)
        phase_c0_piece(pat0, 1, 1)
        emit_b(3)
        phase_c0_piece(pat0, 2, 1)
        emit_q(1)
        phase_c0_piece(pat0, 3, 1)
        phase_b_tr(2)
        phase_b_tr(3)
        osb0 = sb_osb.tile([D + 1, 512], F32, tag="osb", name="osb0")
        nc.scalar.copy(osb0[:], pat0[0 : D + 1, :])
        nc.sync.dma_start(outT[:, 0:512], osb0[:])

        inj_at = {1: [4, 6, 7, 7], 2: [5, 8, 11, 11]}
        for c in range(1, QC):
            if c < QC - 1:
                cn = c + 1
                pts = inj_at[c]
                items = [
                    [mk(emit_b, 2 * cn)],
                    [mk(emit_b, 2 * cn + 1)],
                    [mk(emit_q, cn)],
                    [mk(phase_b_tr, 2 * cn), mk(phase_b_tr, 2 * cn + 1)],
                ]
                inject = {}
                for pt, fns in zip(pts, items):
                    inject.setdefault(pt, []).extend(fns)
            else:
                inject = None
            phase_c(c, inject)

    nc.compile()
    return nc


def _stage_inputs(x, Wq, bq, Wk, bk, Wv, bv):
    import ml_dtypes

    x = np.asarray(x, dtype=np.float32)
    w3 = np.concatenate(
        [np.asarray(Wq), np.asarray(Wk), np.asarray(Wv)], axis=1
    ).astype(ml_dtypes.bfloat16)
    bias2 = np.zeros((P, 2), dtype=np.float32)
    bias2[0:D, 0] = np.asarray(bk, dtype=np.float32)
    bias2[0:D, 1] = np.asarray(bq, dtype=np.float32)

    qv = np.arange(512)
    in_maps = []
    for core in range(N_CORES):
        b, h = divmod(core, 2)
        g = np.arange(NT)
        if h == 1:
            g = g ^ 1
        xb = x[b].reshape(NT, P, E)[g]
        xkT_c = np.ascontiguousarray(
            xb.reshape(S, E).T.astype(ml_dtypes.bfloat16)
        )
        qpos = (P * (2 * (qv // P) + h) + (qv % P)).astype(np.float32)
        qband = np.ascontiguousarray(np.broadcast_to(qpos, (P, 512)))
        kk = np.arange(P)
        jl = np.arange(8)
        kband = (P * (jl[None, :] ^ h) + kk[:, None]).astype(np.float32)
        cst = np.empty((P, 522), dtype=np.float32)
        cst[:, 0:2] = bias2
        cst[:, 2:514] = qband
        cst[:, 514:522] = kband
        in_maps.append(
            {
                "xkT": xkT_c,
                "w3": w3,
                "consts": np.ascontiguousarray(cst),
            }
        )
    return in_maps


def _gather_output(results, bv):
    out = np.empty((B, S, D), dtype=np.float32)
    bv = np.asarray(bv, dtype=np.float32)
    tg = np.array([8 * c + 2 * si for c in range(QC) for si in range(4)])
    for core in range(N_CORES):
        b, h = divmod(core, 2)
        ot = results[core]["outT"]
        attn = ot[0:D] / ot[D : D + 1] + bv[:, None]
        blocks = attn.T.reshape(16, P, D)
        out.reshape(B, NT, P, D)[b, tg + h] = blocks
    return out


def kernel(x, Wq, bq, Wk, bk, Wv, bv):
    if "nc" not in _CACHE:
        _CACHE["nc"] = _build()
    nc = _CACHE["nc"]
    in_maps = _stage_inputs(x, Wq, bq, Wk, bk, Wv, bv)
    res = run_bass_kernel_spmd(nc, in_maps, core_ids=list(range(N_CORES)))
    return _gather_output(res.results, bv)


# revision 7
# speedup vs baseline: 1.0545x; 1.0015x over previous
"""Single-head causal attention (B=4, S=4096, E=512, D=64) on 8 trn2 cores.

v3 (mixed precision): bf16 projections + bf16 scores (exact-ish), fp8
DoubleRow attend with hi/lo-split V, exp split across ACT and Pool.

Sharding: 8 cores = 4 batches x 2 query-interleave groups (as v1): core
(b, h) computes batch b, query tiles {h, h+2, ...}. Host permutes key
tiles per core (pair-swap for h=1) so the SPMD program is slot-identical;
exact causality in the diagonal band uses a 0/1 fp8 mask (Pool engine).

Datapath:
  - x^T, W in bf16; QKV projections as v1 (bf16 matmuls, f32 psum);
    K^T|V^T -> one bf16 SBUF tile (DVE bias-add copy), Q^T -> bf16.
  - scores = K_j^T . Q^T in bf16 per key block (f32 psum), exact.
  - softmax exp -> fp8e4m3 (e^(s/8 - 2); the -2 bias cancels in the
    division and keeps fp8 in range). ACT computes the band pairs + odd
    non-band pairs; Pool (gpsimd pow(e, y)) computes even non-band pairs
    from a DVE-staged copy (fused *0.125 - 2).
  - attend: fp8 DoubleRow, 2 instructions per 256-key block pair:
    vo_h = fp8(V) (+ ones column for the denominator), vo_l =
    fp8(V - vo_h) (the fp8 residual is small, so storing it unscaled in
    fp8 keeps ~0.1% absolute accuracy); both accumulate into one [80,512]
    psum = [numerator; denominator; pad]. Host divides + adds bv.
"""

import numpy as np
from contextlib import ExitStack

import concourse.mybir as mybir
import concourse.tile as tile
from concourse import bacc
from concourse.bass_utils import run_bass_kernel_spmd
from concourse.masks import make_identity

F32 = mybir.dt.float32
BF16 = mybir.dt.bfloat16
FP8 = mybir.dt.float8e4
AF = mybir.ActivationFunctionType
OP = mybir.AluOpType
DR = mybir.MatmulPerfMode.DoubleRow

B, S, E, D = 4, 4096, 512, 64
P = 128
EO = E // P
NT = S // P
KC = S // 512
QC = (S // 2) // 512
N_CORES = 8
ESC = 0.125
EBIAS = -2.0
VOW = 80  # fp8 DoubleRow stationary width (needs % 16 == 0)

_CACHE: dict = {}


def _build():
    nc = bacc.Bacc(
        "TRN2", target_bir_lowering=False, debug=False, num_devices=N_CORES
    )
    xkT = nc.dram_tensor("xkT", [E, S], BF16, kind="ExternalInput").ap()
    w3 = nc.dram_tensor("w3", [E, 3 * D], BF16, kind="ExternalInput").ap()
    consts = nc.dram_tensor("consts", [P, 522], F32, kind="ExternalInput").ap()
    outT = nc.dram_tensor("outT", [D + 1, S // 2], F32, kind="ExternalOutput").ap()

    with tile.TileContext(nc) as tc, ExitStack() as ctx:
        sb_const = ctx.enter_context(tc.tile_pool(name="const", bufs=1))
        sb_kv = ctx.enter_context(tc.tile_pool(name="kv", bufs=1))
        sb_xk = ctx.enter_context(tc.tile_pool(name="xk", bufs=8))
        sb_exp = ctx.enter_context(tc.tile_pool(name="exp", bufs=12))
        sb_sc = ctx.enter_context(tc.tile_pool(name="scb", bufs=4))
        sb_osb = ctx.enter_context(tc.tile_pool(name="osb", bufs=2))
        ps_misc = ctx.enter_context(tc.tile_pool(name="psm", bufs=1, space="PSUM"))
        ps_sc = ctx.enter_context(tc.tile_pool(name="pssc", bufs=3, space="PSUM"))
        ps_at = ctx.enter_context(tc.tile_pool(name="psat", bufs=1, space="PSUM"))
        
        # ---------------- constants ----------------
        w3t = sb_const.tile([P, EO, 3 * D], BF16)
        nc.scalar.dma_start(w3t[:], w3.rearrange("(eo p) d -> p eo d", p=P))
        cst = sb_const.tile([P, 522], F32)
        nc.scalar.dma_start(cst[:], consts)
        b2 = cst[:, 0:2]
        qb = cst[:, 2:514]
        kb = cst[:, 514:522]
        ebase = sb_const.tile([P, 1], F32)
        nc.gpsimd.memset(ebase[:], float(np.e))
        ebias = sb_const.tile([P, 1], F32)
        nc.gpsimd.memset(ebias[:], EBIAS)
        identF = sb_const.tile([P, P], F32)
        make_identity(nc, identF[:])
        identB = sb_const.tile([P, P], BF16)
        nc.vector.tensor_copy(identB[:], identF[:])
        bmask = sb_const.tile([P, 8, P], FP8)

        def build_masks():
            for jl in range(8):
                m = jl // 2
                qc0 = 128 * m
                nc.vector.tensor_tensor(
                    out=bmask[:, jl, :],
                    in0=qb[:, qc0 : qc0 + P],
                    in1=kb[:, jl : jl + 1].to_broadcast((P, P)),
                    op=OP.is_ge,
                )

        # ---------------- persistent state ----------------
        # kv rows 0:64 = K^T + bk, rows 64:128 = V^T (bf16)
        kv = sb_kv.tile([P, S], BF16)
        qts = sb_kv.tile([D, S // 2], BF16)
        # fp8 V blocks in [k, d] layout: hi + ones column, lo residual
        vo_h = sb_kv.tile([P, NT, VOW], FP8)
        vo_l = sb_kv.tile([P, NT, VOW], FP8)
        nc.gpsimd.memset(vo_h[:, :, D:VOW], 0.0)
        nc.gpsimd.memset(vo_h[:, :, D], 1.0)
        nc.gpsimd.memset(vo_l[:, :, D:VOW], 0.0)

        def phase_b(kc):
            xk = sb_xk.tile([P, EO, 512], BF16, tag="xk", name=f"xk{kc}")
            src = xkT[:, kc * 512 : (kc + 1) * 512].rearrange(
                "(eo p) k -> p eo k", p=P
            )
            if kc == 0:
                # two half-chunk pieces so the first projection starts early
                nc.sync.dma_start(xk[:, 0:2, :], src[:, 0:2, :])
                nc.sync.dma_start(xk[:, 2:4, :], src[:, 2:4, :])
            else:
                nc.sync.dma_start(xk[:], src)
            pkv = ps_misc.tile([P, 512], F32, tag="ps", name=f"pkv{kc}")
            for eo in range(EO):
                nc.tensor.matmul(
                    pkv[:],
                    w3t[:, eo, D : 3 * D],
                    xk[:, eo, :],
                    start=(eo == 0),
                    stop=(eo == EO - 1),
                )
            nc.vector.tensor_tensor(
                out=kv[:, kc * 512 : (kc + 1) * 512],
                in0=pkv[:],
                in1=b2[:, 0:1].to_broadcast((P, 512)),
                op=OP.add,
            )
            return xk

        def phase_b_tr(kc):
            # V^T -> V transposes (4 key blocks, bf16), then split into
            # fp8 hi + fp8 residual lo
            pt = ps_misc.tile([P, 512], F32, tag="ps", name=f"pt{kc}")
            ptv = pt[:, 0:P].bitcast(BF16).rearrange(
                "p (b x) -> p b x", b=4
            )
            for bb in range(4):
                j = 4 * kc + bb
                nc.tensor.transpose(
                    ptv[:, bb, :],
                    kv[D:P, j * P : (j + 1) * P],
                    identB[D:P, D:P],
                )
            nc.vector.tensor_copy(
                vo_h[:, 4 * kc : 4 * kc + 4, 0:D], ptv[:]
            )
            nc.vector.tensor_tensor(
                out=vo_l[:, 4 * kc : 4 * kc + 4, 0:D],
                in0=ptv[:],
                in1=vo_h[:, 4 * kc : 4 * kc + 4, 0:D],
                op=OP.subtract,
            )

        pq_tiles = {}

        def phase_b_q_proj(c, xk_h, half):
            if half == 0:
                pq_tiles[c] = ps_sc.tile([P, 512], F32, tag="sc", name=f"pq{c}")
            pq = pq_tiles[c]
            for eo in range(EO):
                rhs = xk_h[:, eo, :].rearrange(
                    "p (t2 two x) -> p t2 two x", two=2, x=P
                )[:, :, 0, :]
                nc.tensor.matmul(
                    pq[0:D, half * 256 : (half + 1) * 256],
                    w3t[:, eo, 0:D],
                    rhs,
                    start=(eo == 0),
                    stop=(eo == EO - 1),
                )

        def phase_b_q_copy(c, half):
            pq = pq_tiles[c]
            nc.scalar.activation(
                qts[:, c * 512 + half * 256 : c * 512 + (half + 1) * 256],
                pq[0:D, half * 256 : (half + 1) * 256],
                AF.Identity,
                bias=b2[0:D, 1:2],
            )

        def phase_b_q_half(c, xk_h, half):
            phase_b_q_proj(c, xk_h, half)
            phase_b_q_copy(c, half)

        def phase_b_q(c, xk_a, xk_b):
            phase_b_q_proj(c, xk_a, 0)
            phase_b_q_proj(c, xk_b, 1)
            phase_b_q_copy(c, 0)
            phase_b_q_copy(c, 1)

        def emit_scores(psc, c, p2, qs):
            for ji, j in ((0, 2 * p2), (1, 2 * p2 + 1)):
                nc.tensor.matmul(
                    psc[:, 512 * ji + qs : 512 * ji + 512],
                    kv[0:D, j * P : (j + 1) * P],
                    qts[:, c * 512 + qs : (c + 1) * 512],
                    start=True,
                    stop=True,
                )

        def emit_exp_act(psc, eT, qe, width=512):
            psc_v = psc[:].rearrange("p (two x) -> p two x", x=512)
            eT_v = eT[:].rearrange("p (two x) -> p two x", x=512)
            nc.scalar.activation(
                eT_v[:, :, qe:width], psc_v[:, :, qe:width], AF.Exp,
                scale=ESC, bias=ebias[:, 0:1],
            )

        def emit_exp_pool(psc, eT, name):
            scb = sb_sc.tile([P, 1024], F32, tag="scb", name=name)
            for hh in range(2):
                sl = slice(512 * hh, 512 * hh + 512)
                nc.vector.tensor_scalar(
                    out=scb[:, sl], in0=psc[:, sl], scalar1=ESC, scalar2=EBIAS,
                    op0=OP.mult, op1=OP.add,
                )
                nc.gpsimd.tensor_tensor(
                    out=eT[:, sl], in0=ebase[:, 0:1].to_broadcast((P, 512)),
                    in1=scb[:, sl], op=OP.pow,
                )

        def emit_mask(eT, m, qe):
            eT_v = eT[:].rearrange("p (two x) -> p two x", x=512)
            nc.gpsimd.tensor_tensor(
                out=eT_v[:, :, qe : qe + P],
                in0=eT_v[:, :, qe : qe + P],
                in1=bmask[:, 2 * m : 2 * m + 2, :],
                op=OP.mult,
            )

        def emit_av(pat, eT, p2, qe, start, stop, width=512):
            eT_v = eT[:].rearrange("p (two x) -> p two x", x=512)
            nc.tensor.matmul(
                pat[:, qe:width],
                vo_h[:, 2 * p2 : 2 * p2 + 2, :],
                eT_v[:, :, qe:width],
                start=start,
                stop=False,
                perf_mode=DR,
            )
            nc.tensor.matmul(
                pat[:, qe:width],
                vo_l[:, 2 * p2 : 2 * p2 + 2, :],
                eT_v[:, :, qe:width],
                start=False,
                stop=stop,
                perf_mode=DR,
            )


        def phase_c(c, inject=None):
            AVLAG = 8 if c < QC - 1 else 6
            pat = ps_at.tile([VOW, 512], F32, tag="at", name=f"at{c}")
            npair = 4 * c + 4
            order = list(range(4 * c, npair)) + list(range(0, 4 * c))
            pend = []  # (eT, p2, qe) awaiting their AV emission
            navs = [0]

            def flush_av(last):
                eT, p2, qe = pend.pop(0)
                emit_av(pat, eT, p2, qe, start=(navs[0] == 0), stop=last)
                navs[0] += 1

            for idx, p2 in enumerate(order):
                if inject and idx in inject:
                    for fn in inject[idx]:
                        fn()
                m = p2 - 4 * c
                qs = 0 if m < 0 else min(128 * m, 256)
                qe = 0 if m < 0 else 128 * m
                psc = ps_sc.tile([P, 1024], F32, tag="sc", name=f"sc{c}_{p2}")
                emit_scores(psc, c, p2, qs)
                eT = sb_exp.tile([P, 1024], FP8, tag="eT", name=f"eT{c}_{p2}")
                if m < 0 and p2 % 2 == 1 and not (c == 3 and p2 >= 10):
                    emit_exp_pool(psc, eT, f"scb{c}_{p2}")
                else:
                    emit_exp_act(psc, eT, qe)
                if m >= 0:
                    emit_mask(eT, m, qe)
                pend.append((eT, p2, qe))
                if len(pend) > AVLAG:
                    flush_av(False)
            while pend:
                flush_av(len(pend) == 1)
            osb = sb_osb.tile([D + 1, 512], F32, tag="osb", name=f"osb{c}")
            nc.scalar.copy(osb[:], pat[0 : D + 1, :])
            nc.sync.dma_start(outT[:, c * 512 : (c + 1) * 512], osb[:])

        # ---------------- schedule ----------------
        xk_tiles = {}

        def mk(fn, *args):
            return lambda: fn(*args)

        def emit_b(kc):
            xk_tiles[kc] = phase_b(kc)

        def emit_q(c):
            phase_b_q(c, xk_tiles[2 * c], xk_tiles[2 * c + 1])

        def phase_c0_piece(pat0, m, h):
            qe = 128 * m
            a, b = max(qe, 256 * h), 256 * h + 256
            if b - a <= 0:
                return
            psc = ps_sc.tile([P, 1024], F32, tag="sc", name=f"s0_{m}_{h}")
            for ji, j in ((0, 2 * m), (1, 2 * m + 1)):
                nc.tensor.matmul(
                    psc[:, 512 * ji + a : 512 * ji + b],
                    kv[0:D, j * P : (j + 1) * P],
                    qts[:, a:b],
                    start=True,
                    stop=True,
                )
            eT = sb_exp.tile([P, 1024], FP8, tag="eT", name=f"e0_{m}_{h}")
            psc_v = psc[:].rearrange("p (two x) -> p two x", x=512)
            eT_v = eT[:].rearrange("p (two x) -> p two x", x=512)
            nc.scalar.activation(
                eT_v[:, :, a:b], psc_v[:, :, a:b], AF.Exp, scale=ESC,
                bias=ebias[:, 0:1],
            )
            if a <= qe < b:
                emit_mask(eT, m, qe)
            emit_av(pat0, eT, m, a, start=(m == 0), stop=(m == 3 and h == 1), width=b)

        # chunk-0 split head schedule
        emit_b(0)
        build_masks()
        phase_b_q_half(0, xk_tiles[0], 0)
        phase_b_tr(0)
        pat0 = ps_at.tile([VOW, 512], F32, tag="at", name="at0")
        phase_c0_piece(pat0, 0, 0)
        phase_c0_piece(pat0, 1, 0)
        emit_b(1)
        phase_b_q_half(0, xk_tiles[1], 1)
        phase_b_tr(1)
        phase_c0_piece(pat0, 0, 1)
        emit_b(2)
        phase_c0_piece(pat0, 1, 1)
        emit_b(3)
        phase_c0_piece(pat0, 2, 1)
        emit_q(1)
        phase_c0_piece(pat0, 3, 1)
        phase_b_tr(2)
        phase_b_tr(3)
        osb0 = sb_osb.tile([D + 1, 512], F32, tag="osb", name="osb0")
        nc.scalar.copy(osb0[:], pat0[0 : D + 1, :])
        nc.sync.dma_start(outT[:, 0:512], osb0[:])

        inj_at = {1: [4, 6, 7, 7], 2: [5, 8, 11, 11]}
        for c in range(1, QC):
            if c < QC - 1:
                cn = c + 1
                pts = inj_at[c]
                items = [
                    [mk(emit_b, 2 * cn)],
                    [mk(emit_b, 2 * cn + 1)],
                    [mk(emit_q, cn)],
                    [mk(phase_b_tr, 2 * cn), mk(phase_b_tr, 2 * cn + 1)],
                ]
                inject = {}
                for pt, fns in zip(pts, items):
                    inject.setdefault(pt, []).extend(fns)
            else:
                inject = None
            phase_c(c, inject)

    nc.compile()
    return nc


def _stage_inputs(x, Wq, bq, Wk, bk, Wv, bv):
    import ml_dtypes

    x = np.asarray(x, dtype=np.float32)
    w3 = np.concatenate(
        [np.asarray(Wq), np.asarray(Wk), np.asarray(Wv)], axis=1
    ).astype(ml_dtypes.bfloat16)
    bias2 = np.zeros((P, 2), dtype=np.float32)
    bias2[0:D, 0] = np.asarray(bk, dtype=np.float32)
    bias2[0:D, 1] = np.asarray(bq, dtype=np.float32)

    qv = np.arange(512)
    in_maps = []
    for core in range(N_CORES):
        b, h = divmod(core, 2)
        g = np.arange(NT)
        if h == 1:
            g = g ^ 1
        xb = x[b].reshape(NT, P, E)[g]
        xkT_c = np.ascontiguousarray(
            xb.reshape(S, E).T.astype(ml_dtypes.bfloat16)
        )
        qpos = (P * (2 * (qv // P) + h) + (qv % P)).astype(np.float32)
        qband = np.ascontiguousarray(np.broadcast_to(qpos, (P, 512)))
        kk = np.arange(P)
        jl = np.arange(8)
        kband = (P * (jl[None, :] ^ h) + kk[:, None]).astype(np.float32)
        cst = np.empty((P, 522), dtype=np.float32)
        cst[:, 0:2] = bias2
        cst[:, 2:514] = qband
        cst[:, 514:522] = kband
        in_maps.append(
            {
                "xkT": xkT_c,
                "w3": w3,
                "consts": np.ascontiguousarray(cst),
            }
        )
    return in_maps


def _gather_output(results, bv):
    out = np.empty((B, S, D), dtype=np.float32)
    bv = np.asarray(bv, dtype=np.float32)
    tg = np.array([8 * c + 2 * si for c in range(QC) for si in range(4)])
    for core in range(N_CORES):
        b, h = divmod(core, 2)
        ot = results[core]["outT"]
        attn = ot[0:D] / ot[D : D + 1] + bv[:, None]
        blocks = attn.T.reshape(16, P, D)
        out.reshape(B, NT, P, D)[b, tg + h] = blocks
    return out


def kernel(x, Wq, bq, Wk, bk, Wv, bv):
    if "nc" not in _CACHE:
        _CACHE["nc"] = _build()
    nc = _CACHE["nc"]
    in_maps = _stage_inputs(x, Wq, bq, Wk, bk, Wv, bv)
    res = run_bass_kernel_spmd(nc, in_maps, core_ids=list(range(N_CORES)))
    return _gather_output(res.results, bv)
